# revision 20
# baseline (speedup 1.0000x reference)
"""BitNet attention block on 8 TRN2 NeuronCores (tensor-parallel over heads).

v2: bf16 datapath. Weights are stored as PURE ternary {-1,0,+1} bf16; all
absmean gammas are folded into three scalar application points (the exp's
input scale c = gq*gk/sqrt(HD), the mask pre-scale 1/c, and the o_proj
output scale gv*go). Quantization streams the f32 weights twice from HBM
(sum pass for gamma, quant pass) through a small rotating SBUF buffer, so
phase-1 matmuls start as soon as the first weight chunk is quantized.
DMAs are issued as single multi-tile instructions over rearranged
[128, kt, col] views so descriptors spread across all 16 DMA engines.
Partition reductions/broadcasts run on the (otherwise idle) GpSimd engine.
AllGather is chunked per (batch, half) in bf16 and o_proj chunks are
interleaved with the second batch's attention.

Sharding: core c owns Q heads [4c,4c+4), KV head c, o_proj output dims
[512c, 512c+512). Host does layout transforms (transpose/shard/cast) only.
"""
import os
import sys
sys.path.insert(0, "/opt/trn_rl_repo")
import numpy as np
import ml_dtypes

BFNP = ml_dtypes.bfloat16
B, S, H = 2, 1024, 4096
NH, NKV, HD = 32, 8, 128
NCORES = 8
T = B * S
QH = NH // NCORES          # 4 q-heads per core
MSH = H // NCORES          # 512 o_proj out-dims per core
THETA = 10000.0
C_MAGIC = 12582912.0       # 1.5 * 2**23: (x + C) - C == round-half-even(x)
TWO_PI = 6.283185307179586
NKT = H // 128             # 32 contraction tiles
NTC = T // 512             # 4 token chunks
SKT = S // 128             # 8 score k-tiles per batch
SQC = S // 512             # 2 q-chunks per batch
WCH = 8                    # kt per wq/wo quant chunk (4 chunks each)
XCH = 8                    # kt per x chunk (4 chunks per tcn)

NQ = float(NH * HD * H)
NK = float(NKV * HD * H)
NO = float(H * NH * HD)

_cache = {}
last_exec_time_ns = None


def _classify_mask(mask):
    """Per (b, kt, qc) [128k x 512q] block: 0 zero, 1 masked-out, 2 general."""
    status = np.empty((B, SKT, SQC), dtype=np.int8)
    index = {}
    packed = []
    for b in range(B):
        mb = np.asarray(mask[b, 0], dtype=np.float32)   # (q, k)
        for kt in range(SKT):
            for qc in range(SQC):
                blk = mb[qc * 512:(qc + 1) * 512, kt * 128:(kt + 1) * 128]
                if not blk.any():
                    status[b, kt, qc] = 0
                elif (blk <= -1e4).all():
                    status[b, kt, qc] = 1
                else:
                    status[b, kt, qc] = 2
                    index[(b, kt, qc)] = len(packed)
                    packed.append(np.ascontiguousarray(blk.T))  # (128k, 512q)
    if packed:
        packed_arr = np.concatenate(packed, axis=0)
    else:
        packed_arr = np.zeros((128, 512), dtype=np.float32)
    return status, index, packed_arr.astype(BFNP)


def _cody_consts():
    c1 = float(np.float32(6.28125))
    r = np.float64(TWO_PI) - c1
    c2 = float(np.float32(r - np.remainder(r, 2.0 ** -24)))
    c3 = float(np.float32(np.float64(TWO_PI) - c1 - float(c2)))
    return c1, c2, c3


def _build(status, index, n_packed):
    from concourse import bacc, tile, mybir, bass_isa

    F32 = mybir.dt.float32
    BF16 = mybir.dt.bfloat16
    ACTF = mybir.ActivationFunctionType
    ALU = mybir.AluOpType
    X = mybir.AxisListType.X
    RADD = bass_isa.ReduceOp.add
    RG = [list(range(NCORES))]
    c1, c2, c3 = _cody_consts()

    nc = bacc.Bacc("TRN2", target_bir_lowering=False, debug=False,
                   num_devices=NCORES)

    xT = nc.dram_tensor("xT", [H, T], BF16, kind="ExternalInput")
    wqT = nc.dram_tensor("wqT", [H, QH * HD], F32, kind="ExternalInput")
    wkT = nc.dram_tensor("wkT", [H, HD], F32, kind="ExternalInput")
    wvT = nc.dram_tensor("wvT", [H, HD], F32, kind="ExternalInput")
    woT = nc.dram_tensor("woT", [H, MSH], F32, kind="ExternalInput")
    maskP = nc.dram_tensor("maskP", [n_packed * 128, 512], BF16,
                           kind="ExternalInput")
    pos = nc.dram_tensor("pos", [1, T], F32, kind="ExternalInput")
    outN = nc.dram_tensor("outN", [T, MSH], F32, kind="ExternalOutput")

    # [128, kt, col] views: element (p, k, c) = tensor[128k + p, c]
    wqR = wqT[:, :].rearrange("(k p) c -> p k c", p=128)
    wkR = wkT[:, :].rearrange("(k p) c -> p k c", p=128)
    wvR = wvT[:, :].rearrange("(k p) c -> p k c", p=128)
    woR = woT[:, :].rearrange("(k p) c -> p k c", p=128)
    xR = xT[:, :].rearrange("(k p) c -> p k c", p=128)
    mR = maskP[:, :].rearrange("(k p) c -> p k c", p=128)

    idn_c = nc.inline_tensor(np.eye(128, dtype=np.float32), name="idn_c")
    invf_np = (1.0 / THETA ** (np.arange(0, HD, 2, dtype=np.float32) / HD))
    invf_np = np.concatenate([invf_np, invf_np]).reshape(HD, 1)
    invf_c = nc.inline_tensor(invf_np.astype(np.float32), name="invf_c")

    with tile.TileContext(nc) as tc, \
         nc.allow_low_precision(reason="bf16 kernel"):
        with tc.tile_pool(name="cpool", bufs=1) as cpool, \
             tc.tile_pool(name="dbounce", bufs=1, space="DRAM") as dbounce:
            # DRAM bounce tiles (tracked by Tile for collective deps)
            ar_in = dbounce.tile([1, 8], F32, name="ar_in")
            ar_out = dbounce.tile([1, 8], F32, name="ar_out",
                                  addr_space="Shared")
            ar2_in = dbounce.tile([1, 8], F32, name="ar2_in")
            ar2_out = dbounce.tile([1, 8], F32, name="ar2_out",
                                   addr_space="Shared")
            agin = [[dbounce.tile([QH * HD, 512], BF16, name=f"agin{b}_{qc}")
                     for qc in range(SQC)] for b in range(B)]
            agout = [[dbounce.tile([H, 512], BF16, name=f"agout{b}_{qc}",
                                   addr_space="Shared") for qc in range(SQC)]
                     for b in range(B)]
            agoutR = [[agout[b][qc][:, :].rearrange("(k p) c -> p k c", p=128)
                       for qc in range(SQC)] for b in range(B)]

            idn_f = cpool.tile([128, 128], F32, name="idn_f")
            nc.sync.dma_start(out=idn_f[:], in_=idn_c[:, :])
            idn = cpool.tile([128, 128], BF16, name="idn")
            nc.scalar.copy(idn[:], idn_f[:])
            onesk = cpool.tile([128, 1], BF16, name="onesk")
            nc.vector.memset(onesk[:], 1.0)
            invf = cpool.tile([128, 1], F32, name="invf")
            nc.sync.dma_start(out=invf[:], in_=invf_c[:, :])
            cmag = cpool.tile([128, 1], F32, name="cmag")
            nc.vector.memset(cmag[:], C_MAGIC)
            # broadcast scalar columns:
            #   bsc: 0=1/gq 1=1/gk 2=1/gv 3=c 4=1/c
            #   bsc2: 0=1/go 1=gv*go
            bsc = cpool.tile([128, 8], F32, name="bsc")
            bsc2 = cpool.tile([128, 2], F32, name="bsc2")

            # ---------------- pool allocation (LIFO by release time) ----
            # bottom: live until the end
            wop = tc.alloc_tile_pool(name="wop", bufs=1)
            wo_sb = wop.tile([128, NKT, MSH], BF16, name="wo_sb")
            qkvp = tc.alloc_tile_pool(name="qkvp", bufs=1)
            qT_sb = [qkvp.tile([128, T], BF16, name=f"qT{h}")
                     for h in range(QH)]
            kT_sb = qkvp.tile([128, T], BF16, name="kT_sb")
            vT_sb = qkvp.tile([128, T], BF16, name="vT_sb")
            mall_p = tc.alloc_tile_pool(name="mall_p", bufs=1)
            mall = mall_p.tile([128, n_packed, 512], BF16, name="mall")
            gacc = tc.alloc_tile_pool(name="gacc", bufs=1)
            qscr = tc.alloc_tile_pool(name="qscr", bufs=4)
            # released at end of phase 1 (LIFO: xpool, wstream, wqq_p, tabp)
            tabp = tc.alloc_tile_pool(name="tabp", bufs=1)
            cos_sb = tabp.tile([128, T], BF16, name="cos_sb")
            ss_sb = tabp.tile([128, T], BF16, name="ss_sb")

            # ---------------- RoPE tables (scratch freed before weights)
            with tc.tile_pool(name="rtab", bufs=2) as rtab:
                for tcn in range(NTC):
                    cs = slice(tcn * 512, (tcn + 1) * 512)
                    pchunk = rtab.tile([1, 512], F32, name=f"pos{tcn}",
                                       tag="pos")
                    nc.sync.dma_start(out=pchunk[:], in_=pos[0:1, cs])
                    pf = rtab.tile([128, 512], F32, name=f"pf{tcn}", tag="pf")
                    nc.gpsimd.partition_broadcast(pf[:], pchunk[:],
                                                  channels=128)
                    f_sb = rtab.tile([128, 512], F32, name=f"f{tcn}", tag="f")
                    nc.scalar.activation(f_sb[:], pf[:], ACTF.Copy,
                                         scale=invf[:])
                    k_sb = rtab.tile([128, 512], F32, name=f"kk{tcn}",
                                     tag="kk")
                    nc.vector.tensor_scalar(k_sb[:], f_sb[:], 1.0 / TWO_PI,
                                            C_MAGIC, ALU.mult, ALU.add)
                    nc.vector.tensor_scalar(k_sb[:], k_sb[:], C_MAGIC, None,
                                            ALU.subtract)
                    y_sb = rtab.tile([128, 512], F32, name=f"y{tcn}", tag="y")
                    nc.vector.scalar_tensor_tensor(
                        y_sb[:], k_sb[:], -c1, f_sb[:], ALU.mult, ALU.add)
                    nc.vector.scalar_tensor_tensor(
                        y_sb[:], k_sb[:], -c2, y_sb[:], ALU.mult, ALU.add)
                    nc.vector.scalar_tensor_tensor(
                        y_sb[:], k_sb[:], -c3, y_sb[:], ALU.mult, ALU.add)
                    nc.scalar.activation(ss_sb[0:64, cs], y_sb[0:64, :],
                                         ACTF.Sin, scale=-1.0)
                    nc.scalar.activation(ss_sb[64:128, cs], y_sb[64:128, :],
                                         ACTF.Sin)
                    yc = rtab.tile([128, 512], F32, name=f"yc{tcn}", tag="yc")
                    nc.vector.tensor_scalar(yc[:], y_sb[:],
                                            float(np.pi / 2), None, ALU.add)
                    m_sb = rtab.tile([128, 512], F32, name=f"mm{tcn}",
                                     tag="mm")
                    nc.vector.tensor_scalar(m_sb[:], yc[:], float(np.pi),
                                            None, ALU.is_gt)
                    nc.vector.scalar_tensor_tensor(
                        yc[:], m_sb[:], -TWO_PI, yc[:], ALU.mult, ALU.add)
                    nc.scalar.activation(cos_sb[:, cs], yc[:], ACTF.Sin)

            wqq_p = tc.alloc_tile_pool(name="wqq_p", bufs=1)
            wq_sb = wqq_p.tile([128, NKT, QH * HD], BF16, name="wq_sb")
            wk_sb = wqq_p.tile([128, NKT, HD], BF16, name="wk_sb")
            wv_sb = wqq_p.tile([128, NKT, HD], BF16, name="wv_sb")
            wstream = tc.alloc_tile_pool(name="wstream", bufs=2)
            xpool = tc.alloc_tile_pool(name="xpool", bufs=2)

            # ---------------- weight pass 1: |w| sums -------------------
            accq = gacc.tile([128, NKT], F32, name="accq")
            acck = gacc.tile([128, NKT], F32, name="acck")
            accv = gacc.tile([128, NKT], F32, name="accv")
            acco = gacc.tile([128, NKT], F32, name="acco")
            g4 = gacc.tile([128, 4], F32, name="g4")
            g4r = gacc.tile([128, 4], F32, name="g4r")
            go1 = gacc.tile([128, 1], F32, name="go1")
            go1r = gacc.tile([128, 1], F32, name="go1r")

            NWCH = NKT // WCH
            for c in range(NWCH):
                wc = wstream.tile([128, WCH, 512], F32, name=f"p1q{c}",
                                  tag="w")
                nc.sync.dma_start(out=wc[:],
                                  in_=wqR[:, c * WCH:(c + 1) * WCH, :])
                for i in range(WCH):
                    nc.vector.tensor_reduce(
                        accq[:, c * WCH + i:c * WCH + i + 1], wc[:, i, :], X,
                        ALU.add, apply_absolute_value=True)
            wck = wstream.tile([128, NKT, HD], F32, name="p1k", tag="w")
            nc.sync.dma_start(out=wck[:], in_=wkR[:, :, :])
            for i in range(NKT):
                nc.vector.tensor_reduce(acck[:, i:i + 1], wck[:, i, :], X,
                                        ALU.add, apply_absolute_value=True)
            wcv = wstream.tile([128, NKT, HD], F32, name="p1v", tag="w")
            nc.sync.dma_start(out=wcv[:], in_=wvR[:, :, :])
            for i in range(NKT):
                nc.vector.tensor_reduce(accv[:, i:i + 1], wcv[:, i, :], X,
                                        ALU.add, apply_absolute_value=True)
            nc.vector.tensor_reduce(g4[:, 0:1], accq[:], X, ALU.add)
            nc.vector.tensor_reduce(g4[:, 1:2], acck[:], X, ALU.add)
            nc.vector.tensor_reduce(g4[:, 2:3], accv[:], X, ALU.add)
            nc.vector.memset(g4[:, 3:4], 0.0)
            nc.gpsimd.partition_all_reduce(g4r[:], g4[:], channels=128,
                                           reduce_op=RADD)
            gq_sb = gacc.tile([1, 8], F32, name="gq_sb")
            nc.vector.memset(gq_sb[:], 0.0)
            nc.scalar.copy(gq_sb[0:1, 0:4], g4r[0:1, 0:4])
            nc.sync.dma_start(out=ar_in[:], in_=gq_sb[:])
            nc.gpsimd.collective_compute(
                "AllReduce", ALU.add, replica_groups=RG,
                ins=[ar_in[:].opt()], outs=[ar_out[:].opt()])
            arq_sb = gacc.tile([1, 8], F32, name="arq_sb")
            nc.sync.dma_start(out=arq_sb[:], in_=ar_out[:])

            # gamma math on partition 0 lanes
            gval = gacc.tile([1, 4], F32, name="gval")
            nc.vector.tensor_scalar(gval[0:1, 0:1], arq_sb[0:1, 0:1],
                                    1.0 / NQ, 1e-5, ALU.mult, ALU.add)
            nc.vector.tensor_scalar(gval[0:1, 1:3], arq_sb[0:1, 1:3],
                                    1.0 / NK, 1e-5, ALU.mult, ALU.add)
            gall = gacc.tile([1, 8], F32, name="gall")
            nc.vector.memset(gall[:], 0.0)
            nc.vector.reciprocal(gall[0:1, 0:3], gval[0:1, 0:3])
            nc.vector.tensor_mul(gall[0:1, 3:4], gval[0:1, 0:1],
                                 gval[0:1, 1:2])
            nc.vector.tensor_scalar(gall[0:1, 3:4], gall[0:1, 3:4],
                                    float(1.0 / np.sqrt(HD)), None, ALU.mult)
            nc.vector.reciprocal(gall[0:1, 4:5], gall[0:1, 3:4])
            nc.scalar.copy(gall[0:1, 5:6], gval[0:1, 2:3])
            nc.gpsimd.partition_broadcast(bsc[:], gall[:], channels=128)
            IQ, IK, IV = (bsc[:, i:i + 1] for i in range(3))
            CSC, ICS = bsc[:, 3:4], bsc[:, 4:5]

            # ---------------- quantized weights (pure ternary bf16) -----
            def quant_chunk(srcR, dst_sb, c0, nkt, fw, inv_ap, tagn):
                wc2 = wstream.tile([128, nkt * fw // 512, 512]
                                   if fw == 512 else [128, nkt, fw],
                                   F32, name=f"p2{tagn}", tag="w")
                # reshape trick only valid when nkt*fw == WCH*512
                wc2v = wc2[:]
                nc.scalar.dma_start(out=wc2v,
                                    in_=srcR[:, c0:c0 + nkt, :])
                for i in range(nkt):
                    if fw == 512:
                        src = wc2[:, i, :]
                    else:
                        src = wc2[:, i, :]
                    s = qscr.tile([128, fw], F32, name=f"qs{tagn}_{i}",
                                  tag=f"qs{fw}")
                    nc.scalar.activation(s[:], src, ACTF.Identity,
                                         bias=cmag[:], scale=inv_ap)
                    nc.vector.tensor_scalar(s[:], s[:], C_MAGIC, 1.0,
                                            ALU.subtract, ALU.min)
                    eng = nc.gpsimd if i % 2 == 0 else nc.vector
                    eng.tensor_scalar(dst_sb[:, c0 + i, :], s[:], -1.0,
                                      None, ALU.max)

            quant_chunk(wqR, wq_sb, 0, WCH, 512, IQ, "q0")
            quant_chunk(wkR, wk_sb, 0, NKT, HD, IK, "k")
            quant_chunk(wvR, wv_sb, 0, NKT, HD, IV, "v")
            for c in range(1, NWCH):
                quant_chunk(wqR, wq_sb, c * WCH, WCH, 512, IQ, f"q{c}")

            # ---------------- masks (scaled by 1/c) ---------------------
            nc.gpsimd.dma_start(out=mall[:], in_=mR[:, :, :])
            for mi in range(n_packed):
                nc.scalar.activation(mall[:, mi, :], mall[:, mi, :],
                                     ACTF.Copy, scale=ICS)

            # ---------------- phase 1: QKV projections + RoPE -----------
            NXC = NKT // XCH

            # wo pass 1 tiles interleaved into phase 1 below
            def wo_pass1(c):
                wco = wstream.tile([128, WCH, 512], F32, name=f"p1o{c}",
                                   tag="w")
                nc.sync.dma_start(out=wco[:],
                                  in_=woR[:, c * WCH:(c + 1) * WCH, :])
                for i in range(WCH):
                    nc.vector.tensor_reduce(
                        acco[:, c * WCH + i:c * WCH + i + 1], wco[:, i, :],
                        X, ALU.add, apply_absolute_value=True)

            with tc.tile_pool(name="rope", bufs=2) as rope, \
                 tc.tile_pool(name="p1", bufs=8, space="PSUM") as p1:
                for tcn in range(NTC):
                    cs = slice(tcn * 512, (tcn + 1) * 512)
                    pq = [p1.tile([128, 512], F32, name=f"pq{tcn}_{h}",
                                  tag="p1") for h in range(QH)]
                    pk = p1.tile([128, 512], F32, name=f"pk{tcn}", tag="p1")
                    pv = p1.tile([128, 512], F32, name=f"pv{tcn}", tag="p1")
                    for h2 in range(NXC):
                        xt = xpool.tile([128, XCH, 512], BF16,
                                        name=f"x{tcn}_{h2}", tag="xt")
                        nc.sync.dma_start(
                            out=xt[:],
                            in_=xR[:, h2 * XCH:(h2 + 1) * XCH, cs])
                        for ktl in range(XCH):
                            kt = h2 * XCH + ktl
                            st, sp = (kt == 0), (kt == NKT - 1)
                            for h in range(QH):
                                nc.tensor.matmul(
                                    pq[h][:],
                                    wq_sb[:, kt, h * 128:(h + 1) * 128],
                                    xt[:, ktl, :], start=st, stop=sp,
                                    skip_group_check=True)
                            nc.tensor.matmul(
                                pk[:], wk_sb[:, kt, :], xt[:, ktl, :],
                                start=st, stop=sp, skip_group_check=True)
                            nc.tensor.matmul(
                                pv[:], wv_sb[:, kt, :], xt[:, ktl, :],
                                start=st, stop=sp, skip_group_check=True)

                    def rope_apply(psrc, dst_ap, tg):
                        m1 = rope.tile([128, 512], F32, name=f"m1{tg}",
                                       tag="m1")
                        nc.vector.tensor_mul(m1[:], psrc[:], cos_sb[:, cs])
                        m2 = rope.tile([128, 512], F32, name=f"m2{tg}",
                                       tag="m2")
                        nc.vector.tensor_mul(m2[0:64, :], psrc[64:128, :],
                                             ss_sb[0:64, cs])
                        nc.vector.tensor_mul(m2[64:128, :], psrc[0:64, :],
                                             ss_sb[64:128, cs])
                        nc.vector.tensor_add(dst_ap, m1[:], m2[:])

                    for h in range(QH):
                        rope_apply(pq[h], qT_sb[h][:, cs], f"_{tcn}_{h}")
                    rope_apply(pk, kT_sb[:, cs], f"k_{tcn}")
                    nc.scalar.copy(vT_sb[:, cs], pv[:])
                    # wo gamma pass interleaved (1 chunk per tcn)
                    wo_pass1(tcn)

            # wo gamma: reduce + AllReduce + scalars
            nc.vector.tensor_reduce(go1[:, 0:1], acco[:], X, ALU.add)
            nc.gpsimd.partition_all_reduce(go1r[:], go1[:], channels=128,
                                           reduce_op=RADD)
            go_sb = gacc.tile([1, 8], F32, name="go_sb")
            nc.vector.memset(go_sb[:], 0.0)
            nc.scalar.copy(go_sb[0:1, 0:1], go1r[0:1, 0:1])
            nc.sync.dma_start(out=ar2_in[:], in_=go_sb[:])
            nc.gpsimd.collective_compute(
                "AllReduce", ALU.add, replica_groups=RG,
                ins=[ar2_in[:].opt()], outs=[ar2_out[:].opt()])
            aro_sb = gacc.tile([1, 8], F32, name="aro_sb")
            nc.sync.dma_start(out=aro_sb[:], in_=ar2_out[:])
            gval2 = gacc.tile([1, 2], F32, name="gval2")
            nc.vector.tensor_scalar(gval2[0:1, 0:1], aro_sb[0:1, 0:1],
                                    1.0 / NO, 1e-5, ALU.mult, ALU.add)
            gall2 = gacc.tile([1, 2], F32, name="gall2")
            nc.vector.reciprocal(gall2[0:1, 0:1], gval2[0:1, 0:1])
            nc.vector.tensor_mul(gall2[0:1, 1:2], gval2[0:1, 0:1],
                                 gall[0:1, 5:6])
            nc.gpsimd.partition_broadcast(bsc2[:], gall2[:], channels=128)
            IO, OSC = bsc2[:, 0:1], bsc2[:, 1:2]

            xpool.release()
            wstream.release()
            wqq_p.release()
            tabp.release()

            # ---------------- attention + wo quant + o_proj -------------
            wostream = tc.alloc_tile_pool(name="wostream", bufs=1)

            def wo_quant(c):
                wc2 = wostream.tile([128, WCH, 512], F32, name=f"p2o{c}",
                                    tag="wo")
                nc.scalar.dma_start(out=wc2[:],
                                    in_=woR[:, c * WCH:(c + 1) * WCH, :])
                for i in range(WCH):
                    s = qscr.tile([128, 512], F32, name=f"qso{c}_{i}",
                                  tag="qs512")
                    nc.scalar.activation(s[:], wc2[:, i, :], ACTF.Identity,
                                         bias=cmag[:], scale=IO)
                    nc.vector.tensor_scalar(s[:], s[:], C_MAGIC, 1.0,
                                            ALU.subtract, ALU.min)
                    nc.gpsimd.tensor_scalar(wo_sb[:, c * WCH + i, :], s[:],
                                            -1.0, None, ALU.max)

            with tc.tile_pool(name="vnatp", bufs=2) as vnatp, \
                 tc.tile_pool(name="epool", bufs=16) as epool, \
                 tc.tile_pool(name="zrp", bufs=2) as zrp, \
                 tc.tile_pool(name="zbp", bufs=2) as zbp, \
                 tc.tile_pool(name="aop", bufs=2) as aop, \
                 tc.tile_pool(name="a3", bufs=2) as a3, \
                 tc.tile_pool(name="o3", bufs=2) as o3, \
                 tc.tile_pool(name="ps_s", bufs=2, space="PSUM") as ps_s, \
                 tc.tile_pool(name="ps_po", bufs=2, space="PSUM") as ps_po, \
                 tc.tile_pool(name="ps_z", bufs=1, space="PSUM") as ps_z, \
                 tc.tile_pool(name="ps_tr", bufs=1, space="PSUM") as ps_tr, \
                 tc.tile_pool(name="p3", bufs=2, space="PSUM") as p3:

                def build_vnat(b):
                    boff = b * S
                    vnat = vnatp.tile([128, S], BF16, name=f"vnat{b}",
                                      tag="vnat")
                    for kt in range(SKT):
                        ptr = ps_tr.tile([128, 128], BF16,
                                         name=f"ptr{b}_{kt}", tag="ptr")
                        nc.tensor.transpose(
                            ptr[:],
                            vT_sb[:, boff + kt * 128:boff + (kt + 1) * 128],
                            idn[:])
                        nc.scalar.copy(vnat[:, kt * 128:(kt + 1) * 128],
                                       ptr[:])
                    return vnat

                def attn_unit(b, qc, vnat, hpair):
                    """scores for 2 heads, then softmax+AV for them."""
                    boff = b * S
                    kts = [kt for kt in range(SKT)
                           if status[b, kt, qc] != 1]
                    es = {}
                    for h in hpair:
                        qsl = qT_sb[h][:, boff + qc * 512:
                                       boff + (qc + 1) * 512]
                        for kt in kts:
                            ps_ = ps_s.tile([128, 512], F32,
                                            name=f"s{b}{h}{qc}{kt}",
                                            tag="ps")
                            nc.tensor.matmul(
                                ps_[:],
                                kT_sb[:, boff + kt * 128:
                                      boff + (kt + 1) * 128],
                                qsl, start=True, stop=True,
                                skip_group_check=True)
                            if status[b, kt, qc] == 2:
                                mi = index[(b, kt, qc)]
                                nc.vector.tensor_add(ps_[:], ps_[:],
                                                     mall[:, mi, :])
                            e = epool.tile([128, 512], BF16,
                                           name=f"e{b}{h}{qc}{kt}", tag="e")
                            nc.scalar.activation(e[:], ps_[:], ACTF.Exp,
                                                 scale=CSC)
                            es[(h, kt)] = e
                    for h in hpair:
                        pz = ps_z.tile([1, 512], F32, name=f"pz{b}{h}{qc}",
                                       tag="pz")
                        po = ps_po.tile([128, 512], F32,
                                        name=f"po{b}{h}{qc}", tag="po")
                        for i, kt in enumerate(kts):
                            fst, lst = (i == 0), (i == len(kts) - 1)
                            e = es[(h, kt)]
                            nc.tensor.matmul(pz[:], onesk[:], e[:],
                                             start=fst, stop=lst,
                                             skip_group_check=True)
                            nc.tensor.matmul(
                                po[:], vnat[:, kt * 128:(kt + 1) * 128],
                                e[:], start=fst, stop=lst,
                                skip_group_check=True)
                        zr = zrp.tile([1, 512], BF16, name=f"zr{b}{h}{qc}",
                                      tag="zr")
                        nc.vector.reciprocal(zr[:], pz[:])
                        zb = zbp.tile([128, 512], BF16,
                                      name=f"zb{b}{h}{qc}", tag="zb")
                        nc.gpsimd.partition_broadcast(zb[:], zr[:],
                                                      channels=128)
                        ao = aop.tile([128, 512], BF16, name=f"ao{b}{h}{qc}",
                                      tag="ao")
                        nc.vector.tensor_mul(ao[:], po[:], zb[:])
                        nc.sync.dma_start(
                            out=agin[b][qc][h * 128:(h + 1) * 128, :],
                            in_=ao[:])

                def ag_chunk(b, qc):
                    nc.gpsimd.collective_compute(
                        "AllGather", ALU.bypass, replica_groups=RG,
                        ins=[agin[b][qc][:, :].opt()],
                        outs=[agout[b][qc][:, :].opt()])

                def oproj(b, qc):
                    ch = b * 2 + qc
                    at = a3.tile([128, NKT, 512], BF16, name=f"at{ch}",
                                 tag="at")
                    nc.scalar.dma_start(out=at[:], in_=agoutR[b][qc][:, :, :])
                    for tt in range(4):
                        pout = p3.tile([128, 512], F32, name=f"po3_{ch}{tt}",
                                       tag="pout")
                        for kt in range(NKT):
                            nc.tensor.matmul(
                                pout[:], at[:, kt, tt * 128:(tt + 1) * 128],
                                wo_sb[:, kt, :], start=(kt == 0),
                                stop=(kt == NKT - 1), skip_group_check=True)
                        osb = o3.tile([128, 512], F32, name=f"osb{ch}{tt}",
                                      tag="osb")
                        nc.scalar.activation(osb[:], pout[:], ACTF.Copy,
                                             scale=OSC)
                        nc.sync.dma_start(
                            out=outN[ch * 512 + tt * 128:
                                     ch * 512 + (tt + 1) * 128, :],
                            in_=osb[:])

                # batch 0 attention, wo quant interleaved
                vnat0 = build_vnat(0)
                attn_unit(0, 0, vnat0, (0, 1))
                wo_quant(0)
                attn_unit(0, 0, vnat0, (2, 3))
                ag_chunk(0, 0)
                wo_quant(1)
                attn_unit(0, 1, vnat0, (0, 1))
                wo_quant(2)
                attn_unit(0, 1, vnat0, (2, 3))
                ag_chunk(0, 1)
                wo_quant(3)
                # batch 1 attention with o_proj chunks interleaved
                vnat1 = build_vnat(1)
                attn_unit(1, 0, vnat1, (0, 1))
                attn_unit(1, 0, vnat1, (2, 3))
                ag_chunk(1, 0)
                oproj(0, 0)
                attn_unit(1, 1, vnat1, (0, 1))
                attn_unit(1, 1, vnat1, (2, 3))
                ag_chunk(1, 1)
                oproj(0, 1)
                oproj(1, 0)
                oproj(1, 1)

            wostream.release()
            qscr.release()
            gacc.release()
            mall_p.release()
            qkvp.release()
            wop.release()

    nc.compile()
    return nc


def kernel(hidden_states, Wq, Wk, Wv, Wo, attention_mask, position_ids):
    from concourse.bass_utils import run_bass_kernel_spmd
    from concourse.bass_interp import get_hw_module

    hs = np.ascontiguousarray(np.asarray(hidden_states, dtype=np.float32))
    Wq = np.asarray(Wq, dtype=np.float32)
    Wk = np.asarray(Wk, dtype=np.float32)
    Wv = np.asarray(Wv, dtype=np.float32)
    Wo = np.asarray(Wo, dtype=np.float32)
    mask = np.asarray(attention_mask, dtype=np.float32)
    posf = np.ascontiguousarray(
        np.asarray(position_ids).reshape(1, T).astype(np.float32))

    status, index, packed = _classify_mask(mask)
    n_packed = packed.shape[0] // 128

    key = (status.tobytes(), n_packed)
    if key not in _cache:
        nc = _build(status, index, n_packed)
        nc.m = get_hw_module(nc.m)
        _cache[key] = nc
    nc = _cache[key]

    xTb = np.ascontiguousarray(hs.reshape(T, H).T.astype(BFNP))
    in_maps = []
    for c in range(NCORES):
        in_maps.append({
            "xT": xTb,
            "wqT": np.ascontiguousarray(
                Wq[c * QH * HD:(c + 1) * QH * HD, :].T),
            "wkT": np.ascontiguousarray(Wk[c * HD:(c + 1) * HD, :].T),
            "wvT": np.ascontiguousarray(Wv[c * HD:(c + 1) * HD, :].T),
            "woT": np.ascontiguousarray(Wo[c * MSH:(c + 1) * MSH, :].T),
            "maskP": packed,
            "pos": posf,
        })
    res = run_bass_kernel_spmd(nc, in_maps, core_ids=list(range(NCORES)),
                               trace=bool(os.environ.get("BITNET_TRACE")))
    global last_exec_time_ns
    last_exec_time_ns = res.exec_time_ns
    out = np.concatenate(
        [res.results[c]["outN"] for c in range(NCORES)], axis=1)  # (T, H)
    return np.ascontiguousarray(out).reshape(B, S, H).astype(np.float32)


# revision 28
# speedup vs baseline: 1.1925x; 1.1925x over previous
"""BitNet attention block on 8 TRN2 NeuronCores (tensor-parallel over heads).

v2: bf16 datapath. Weights are stored as PURE ternary {-1,0,+1} bf16; all
absmean gammas are folded into three scalar application points (the exp's
input scale c = gq*gk/sqrt(HD), the mask pre-scale 1/c, and the o_proj
output scale gv*go). Quantization streams the f32 weights twice from HBM
(sum pass for gamma, quant pass) through a small rotating SBUF buffer, so
phase-1 matmuls start as soon as the first weight chunk is quantized.
DMAs are issued as single multi-tile instructions over rearranged
[128, kt, col] views so descriptors spread across all 16 DMA engines.
Partition reductions/broadcasts run on the (otherwise idle) GpSimd engine.
AllGather is chunked per (batch, half) in bf16 and o_proj chunks are
interleaved with the second batch's attention.

Sharding: core c owns Q heads [4c,4c+4), KV head c, o_proj output dims
[512c, 512c+512). Host does layout transforms (transpose/shard/cast) only.
"""
import os
import sys
sys.path.insert(0, "/opt/trn_rl_repo")
import numpy as np
import ml_dtypes

BFNP = ml_dtypes.bfloat16
B, S, H = 2, 1024, 4096
NH, NKV, HD = 32, 8, 128
NCORES = 8
T = B * S
QH = NH // NCORES          # 4 q-heads per core
MSH = H // NCORES          # 512 o_proj out-dims per core
THETA = 10000.0
C_MAGIC = 12582912.0       # 1.5 * 2**23: (x + C) - C == round-half-even(x)
TWO_PI = 6.283185307179586
NKT = H // 128             # 32 contraction tiles
NTC = T // 512             # 4 token chunks
SKT = S // 128             # 8 score k-tiles per batch
SQC = S // 512             # 2 q-chunks per batch
WCH = 8                    # kt per wq/wo quant chunk (4 chunks each)
XCH = 8                    # kt per x chunk (4 chunks per tcn)

NQ = float(NH * HD * H)
NK = float(NKV * HD * H)
NO = float(H * NH * HD)

_cache = {}
last_exec_time_ns = None


def _classify_mask(mask):
    """Per (b, kt, qc) [128k x 512q] block: 0 zero, 1 masked-out, 2 general."""
    status = np.empty((B, SKT, SQC), dtype=np.int8)
    index = {}
    packed = []
    for b in range(B):
        mb = np.asarray(mask[b, 0], dtype=np.float32)   # (q, k)
        for kt in range(SKT):
            for qc in range(SQC):
                blk = mb[qc * 512:(qc + 1) * 512, kt * 128:(kt + 1) * 128]
                if not blk.any():
                    status[b, kt, qc] = 0
                elif (blk <= -1e4).all():
                    status[b, kt, qc] = 1
                else:
                    status[b, kt, qc] = 2
                    index[(b, kt, qc)] = len(packed)
                    packed.append(np.ascontiguousarray(blk.T))  # (128k, 512q)
    if packed:
        packed_arr = np.concatenate(packed, axis=0)
    else:
        packed_arr = np.zeros((128, 512), dtype=np.float32)
    return status, index, packed_arr.astype(BFNP)


def _cody_consts():
    c1 = float(np.float32(6.28125))
    r = np.float64(TWO_PI) - c1
    c2 = float(np.float32(r - np.remainder(r, 2.0 ** -24)))
    c3 = float(np.float32(np.float64(TWO_PI) - c1 - float(c2)))
    return c1, c2, c3


def _build(status, index, n_packed):
    from concourse import bacc, tile, mybir, bass_isa

    F32 = mybir.dt.float32
    BF16 = mybir.dt.bfloat16
    ACTF = mybir.ActivationFunctionType
    ALU = mybir.AluOpType
    X = mybir.AxisListType.X
    RADD = bass_isa.ReduceOp.add
    RG = [list(range(NCORES))]
    c1, c2, c3 = _cody_consts()

    nc = bacc.Bacc("TRN2", target_bir_lowering=False, debug=False,
                   num_devices=NCORES)

    xT = nc.dram_tensor("xT", [H, T], BF16, kind="ExternalInput")
    wqT = nc.dram_tensor("wqT", [H, QH * HD], F32, kind="ExternalInput")
    wkT = nc.dram_tensor("wkT", [H, HD], F32, kind="ExternalInput")
    wvT = nc.dram_tensor("wvT", [H, HD], F32, kind="ExternalInput")
    woT = nc.dram_tensor("woT", [H, MSH], F32, kind="ExternalInput")
    maskP = nc.dram_tensor("maskP", [n_packed * 128, 512], BF16,
                           kind="ExternalInput")
    pos = nc.dram_tensor("pos", [1, T], F32, kind="ExternalInput")
    outN = nc.dram_tensor("outN", [T, MSH], F32, kind="ExternalOutput")

    # [128, kt, col] views: element (p, k, c) = tensor[128k + p, c]
    wqR = wqT[:, :].rearrange("(k p) c -> p k c", p=128)
    wkR = wkT[:, :].rearrange("(k p) c -> p k c", p=128)
    wvR = wvT[:, :].rearrange("(k p) c -> p k c", p=128)
    woR = woT[:, :].rearrange("(k p) c -> p k c", p=128)
    xR = xT[:, :].rearrange("(k p) c -> p k c", p=128)
    mR = maskP[:, :].rearrange("(k p) c -> p k c", p=128)

    idn_c = nc.inline_tensor(np.eye(128, dtype=np.float32), name="idn_c")
    invf_np = (1.0 / THETA ** (np.arange(0, HD, 2, dtype=np.float32) / HD))
    invf_np = np.concatenate([invf_np, invf_np]).reshape(HD, 1)
    invf_c = nc.inline_tensor(invf_np.astype(np.float32), name="invf_c")

    with tile.TileContext(nc) as tc, \
         nc.allow_low_precision(reason="bf16 kernel"):
        with tc.tile_pool(name="cpool", bufs=1) as cpool, \
             tc.tile_pool(name="dbounce", bufs=1, space="DRAM") as dbounce:
            # DRAM bounce tiles (tracked by Tile for collective deps)
            ar_in = dbounce.tile([1, 8], F32, name="ar_in")
            ar_out = dbounce.tile([1, 8], F32, name="ar_out",
                                  addr_space="Shared")
            ar2_in = dbounce.tile([1, 8], F32, name="ar2_in")
            ar2_out = dbounce.tile([1, 8], F32, name="ar2_out",
                                   addr_space="Shared")
            agin = [[dbounce.tile([QH * HD, 512], BF16, name=f"agin{b}_{qc}")
                     for qc in range(SQC)] for b in range(B)]
            agout = [[dbounce.tile([H, 512], BF16, name=f"agout{b}_{qc}",
                                   addr_space="Shared") for qc in range(SQC)]
                     for b in range(B)]
            agoutR = [[agout[b][qc][:, :].rearrange("(k p) c -> p k c", p=128)
                       for qc in range(SQC)] for b in range(B)]

            idn_f = cpool.tile([128, 128], F32, name="idn_f")
            nc.sync.dma_start(out=idn_f[:], in_=idn_c[:, :])
            idn = cpool.tile([128, 128], BF16, name="idn")
            nc.scalar.copy(idn[:], idn_f[:])
            onesk = cpool.tile([128, 1], BF16, name="onesk")
            nc.vector.memset(onesk[:], 1.0)
            invf = cpool.tile([128, 1], F32, name="invf")
            nc.sync.dma_start(out=invf[:], in_=invf_c[:, :])
            cmag = cpool.tile([128, 1], F32, name="cmag")
            nc.vector.memset(cmag[:], C_MAGIC)
            cmag1 = cpool.tile([128, 1], F32, name="cmag1")
            nc.vector.memset(cmag1[:], C_MAGIC + 1.0)
            # broadcast scalar columns:
            #   bsc: 0=1/gq 1=1/gk 2=1/gv 3=c 4=1/c
            #   bsc2: 0=1/go 1=gv*go
            bsc = cpool.tile([128, 8], F32, name="bsc")
            bsc2 = cpool.tile([128, 2], F32, name="bsc2")

            # ---------------- pool allocation (LIFO by release time) ----
            # bottom: live until the end
            wop = tc.alloc_tile_pool(name="wop", bufs=1)
            wo_sb = wop.tile([128, NKT, MSH], BF16, name="wo_sb")
            qkvp = tc.alloc_tile_pool(name="qkvp", bufs=1)
            qT_sb = [qkvp.tile([128, T], BF16, name=f"qT{h}")
                     for h in range(QH)]
            kT_sb = qkvp.tile([128, T], BF16, name="kT_sb")
            vT_sb = qkvp.tile([128, T], BF16, name="vT_sb")
            mall_p = tc.alloc_tile_pool(name="mall_p", bufs=1)
            mall = mall_p.tile([128, n_packed, 512], BF16, name="mall")
            gacc = tc.alloc_tile_pool(name="gacc", bufs=1)
            qscr = tc.alloc_tile_pool(name="qscr", bufs=3)
            # released at end of phase 1 (LIFO: xpool, wstream, wqq_p, tabp)
            tabp = tc.alloc_tile_pool(name="tabp", bufs=1)
            cos_sb = tabp.tile([128, T], BF16, name="cos_sb")
            ss_sb = tabp.tile([128, T], BF16, name="ss_sb")

            # ---------------- RoPE tables (scratch freed before weights)
            with tc.tile_pool(name="rtab", bufs=2) as rtab:
                for tcn in range(NTC):
                    cs = slice(tcn * 512, (tcn + 1) * 512)
                    pchunk = rtab.tile([1, 512], F32, name=f"pos{tcn}",
                                       tag="pos")
                    nc.sync.dma_start(out=pchunk[:], in_=pos[0:1, cs])
                    pf = rtab.tile([128, 512], F32, name=f"pf{tcn}", tag="pf")
                    nc.gpsimd.partition_broadcast(pf[:], pchunk[:],
                                                  channels=128)
                    f_sb = rtab.tile([128, 512], F32, name=f"f{tcn}", tag="f")
                    nc.scalar.activation(f_sb[:], pf[:], ACTF.Copy,
                                         scale=invf[:])
                    k_sb = rtab.tile([128, 512], F32, name=f"kk{tcn}",
                                     tag="kk")
                    nc.vector.tensor_scalar(k_sb[:], f_sb[:], 1.0 / TWO_PI,
                                            C_MAGIC, ALU.mult, ALU.add)
                    nc.vector.tensor_scalar(k_sb[:], k_sb[:], C_MAGIC, None,
                                            ALU.subtract)
                    y_sb = rtab.tile([128, 512], F32, name=f"y{tcn}", tag="y")
                    nc.vector.scalar_tensor_tensor(
                        y_sb[:], k_sb[:], -c1, f_sb[:], ALU.mult, ALU.add)
                    nc.vector.scalar_tensor_tensor(
                        y_sb[:], k_sb[:], -c2, y_sb[:], ALU.mult, ALU.add)
                    nc.vector.scalar_tensor_tensor(
                        y_sb[:], k_sb[:], -c3, y_sb[:], ALU.mult, ALU.add)
                    nc.scalar.activation(ss_sb[0:64, cs], y_sb[0:64, :],
                                         ACTF.Sin, scale=-1.0)
                    nc.scalar.activation(ss_sb[64:128, cs], y_sb[64:128, :],
                                         ACTF.Sin)
                    yc = rtab.tile([128, 512], F32, name=f"yc{tcn}", tag="yc")
                    nc.vector.tensor_scalar(yc[:], y_sb[:],
                                            float(np.pi / 2), None, ALU.add)
                    m_sb = rtab.tile([128, 512], F32, name=f"mm{tcn}",
                                     tag="mm")
                    nc.vector.tensor_scalar(m_sb[:], yc[:], float(np.pi),
                                            None, ALU.is_gt)
                    nc.vector.scalar_tensor_tensor(
                        yc[:], m_sb[:], -TWO_PI, yc[:], ALU.mult, ALU.add)
                    nc.scalar.activation(cos_sb[:, cs], yc[:], ACTF.Sin)

            wqq_p = tc.alloc_tile_pool(name="wqq_p", bufs=1)
            wq_sb = wqq_p.tile([128, NKT, QH * HD], BF16, name="wq_sb")
            wk_sb = wqq_p.tile([128, NKT, HD], BF16, name="wk_sb")
            wv_sb = wqq_p.tile([128, NKT, HD], BF16, name="wv_sb")
            wstream = tc.alloc_tile_pool(name="wstream", bufs=2)
            xpool = tc.alloc_tile_pool(name="xpool", bufs=2)

            # ---------------- weight pass 1: |w| sums -------------------
            accq = gacc.tile([128, NKT], F32, name="accq")
            acck = gacc.tile([128, NKT], F32, name="acck")
            accv = gacc.tile([128, NKT], F32, name="accv")
            acco = gacc.tile([128, NKT], F32, name="acco")
            g4 = gacc.tile([128, 4], F32, name="g4")
            g4r = gacc.tile([128, 4], F32, name="g4r")
            go1 = gacc.tile([128, 1], F32, name="go1")
            go1r = gacc.tile([128, 1], F32, name="go1r")

            NWCH = NKT // WCH
            for c in range(NWCH):
                wc = wstream.tile([128, WCH, 512], F32, name=f"p1q{c}",
                                  tag="w")
                nc.sync.dma_start(out=wc[:],
                                  in_=wqR[:, c * WCH:(c + 1) * WCH, :])
                for i in range(WCH):
                    nc.vector.tensor_reduce(
                        accq[:, c * WCH + i:c * WCH + i + 1], wc[:, i, :], X,
                        ALU.add, apply_absolute_value=True)
            wck = wstream.tile([128, NKT, HD], F32, name="p1k", tag="w")
            nc.sync.dma_start(out=wck[:], in_=wkR[:, :, :])
            for i in range(NKT):
                nc.vector.tensor_reduce(acck[:, i:i + 1], wck[:, i, :], X,
                                        ALU.add, apply_absolute_value=True)
            wcv = wstream.tile([128, NKT, HD], F32, name="p1v", tag="w")
            nc.sync.dma_start(out=wcv[:], in_=wvR[:, :, :])
            for i in range(NKT):
                nc.vector.tensor_reduce(accv[:, i:i + 1], wcv[:, i, :], X,
                                        ALU.add, apply_absolute_value=True)
            nc.vector.tensor_reduce(g4[:, 0:1], accq[:], X, ALU.add)
            nc.vector.tensor_reduce(g4[:, 1:2], acck[:], X, ALU.add)
            nc.vector.tensor_reduce(g4[:, 2:3], accv[:], X, ALU.add)
            nc.vector.memset(g4[:, 3:4], 0.0)
            nc.gpsimd.partition_all_reduce(g4r[:], g4[:], channels=128,
                                           reduce_op=RADD)
            gq_sb = gacc.tile([1, 8], F32, name="gq_sb")
            nc.vector.memset(gq_sb[:], 0.0)
            nc.scalar.copy(gq_sb[0:1, 0:4], g4r[0:1, 0:4])
            nc.sync.dma_start(out=ar_in[:], in_=gq_sb[:])
            nc.gpsimd.collective_compute(
                "AllReduce", ALU.add, replica_groups=RG,
                ins=[ar_in[:].opt()], outs=[ar_out[:].opt()])
            arq_sb = gacc.tile([1, 8], F32, name="arq_sb")
            nc.sync.dma_start(out=arq_sb[:], in_=ar_out[:])

            # gamma math on partition 0 lanes
            gval = gacc.tile([1, 4], F32, name="gval")
            nc.vector.tensor_scalar(gval[0:1, 0:1], arq_sb[0:1, 0:1],
                                    1.0 / NQ, 1e-5, ALU.mult, ALU.add)
            nc.vector.tensor_scalar(gval[0:1, 1:3], arq_sb[0:1, 1:3],
                                    1.0 / NK, 1e-5, ALU.mult, ALU.add)
            gall = gacc.tile([1, 8], F32, name="gall")
            nc.vector.memset(gall[:], 0.0)
            nc.vector.reciprocal(gall[0:1, 0:3], gval[0:1, 0:3])
            nc.vector.tensor_mul(gall[0:1, 3:4], gval[0:1, 0:1],
                                 gval[0:1, 1:2])
            nc.vector.tensor_scalar(gall[0:1, 3:4], gall[0:1, 3:4],
                                    float(1.0 / np.sqrt(HD)), None, ALU.mult)
            nc.vector.reciprocal(gall[0:1, 4:5], gall[0:1, 3:4])
            nc.scalar.copy(gall[0:1, 5:6], gval[0:1, 2:3])
            nc.gpsimd.partition_broadcast(bsc[:], gall[:], channels=128)
            IQ, IK, IV = (bsc[:, i:i + 1] for i in range(3))
            CSC, ICS = bsc[:, 3:4], bsc[:, 4:5]

            # ---------------- quantized weights (pure ternary bf16) -----
            def quant_tile(src, dst, inv_ap, tagn, i):
                """dst_bf16 = clip(round_half_even(src/gamma), -1, 1).

                Round via the +C magic store; clip via Relu (ACT) and min
                (DVE single-op f32). All bf16 conversion on ACT (DVE bf16
                output and dual-op subtract/min paths are slow).
                """
                fw = src.shape[-1]
                t = qscr.tile([128, fw], F32, name=f"qt{tagn}_{i}",
                              tag=f"qa{fw}")
                nc.scalar.activation(t[:], src, ACTF.Identity,
                                     bias=cmag[:], scale=inv_ap)
                r1 = qscr.tile([128, fw], F32, name=f"qr{tagn}_{i}",
                               tag=f"qb{fw}")
                # r1 = max((C+1) - t, 0) = max(1 - u, 0)
                nc.scalar.activation(r1[:], t[:], ACTF.Relu,
                                     bias=cmag1[:], scale=-1.0)
                # m = min(r1, 2) = 1 - tern
                nc.vector.tensor_scalar(r1[:], r1[:], 2.0, None, ALU.min)
                # tern = 1 - m  (bf16 out on ACT)
                nc.scalar.activation(dst, r1[:], ACTF.Copy, bias=1.0,
                                     scale=-1.0)

            def quant_chunk(srcR, dst_sb, c0, nkt, fw, inv_ap, tagn):
                wc2 = wstream.tile([128, nkt, fw], F32, name=f"p2{tagn}",
                                   tag="w")
                nc.scalar.dma_start(out=wc2[:], in_=srcR[:, c0:c0 + nkt, :])
                for i in range(nkt):
                    quant_tile(wc2[:, i, :], dst_sb[:, c0 + i, :], inv_ap,
                               tagn, i)

            quant_chunk(wqR, wq_sb, 0, WCH, 512, IQ, "q0")
            quant_chunk(wkR, wk_sb, 0, NKT, HD, IK, "k")
            quant_chunk(wvR, wv_sb, 0, NKT, HD, IV, "v")
            for c in range(1, NWCH):
                quant_chunk(wqR, wq_sb, c * WCH, WCH, 512, IQ, f"q{c}")

            # ---------------- masks (scaled by 1/c) ---------------------
            nc.gpsimd.dma_start(out=mall[:], in_=mR[:, :, :])
            for mi in range(n_packed):
                nc.scalar.activation(mall[:, mi, :], mall[:, mi, :],
                                     ACTF.Copy, scale=ICS)

            # ---------------- phase 1: QKV projections + RoPE -----------
            NXC = NKT // XCH

            # wo pass 1 tiles interleaved into phase 1 below
            def wo_pass1(c):
                wco = wstream.tile([128, WCH, 512], F32, name=f"p1o{c}",
                                   tag="w")
                nc.sync.dma_start(out=wco[:],
                                  in_=woR[:, c * WCH:(c + 1) * WCH, :])
                for i in range(WCH):
                    nc.vector.tensor_reduce(
                        acco[:, c * WCH + i:c * WCH + i + 1], wco[:, i, :],
                        X, ALU.add, apply_absolute_value=True)

            with tc.tile_pool(name="rope", bufs=2) as rope, \
                 tc.tile_pool(name="p1", bufs=8, space="PSUM") as p1:
                for tcn in range(NTC):
                    cs = slice(tcn * 512, (tcn + 1) * 512)
                    pq = [p1.tile([128, 512], F32, name=f"pq{tcn}_{h}",
                                  tag="p1") for h in range(QH)]
                    pk = p1.tile([128, 512], F32, name=f"pk{tcn}", tag="p1")
                    pv = p1.tile([128, 512], F32, name=f"pv{tcn}", tag="p1")
                    for h2 in range(NXC):
                        xt = xpool.tile([128, XCH, 512], BF16,
                                        name=f"x{tcn}_{h2}", tag="xt")
                        nc.sync.dma_start(
                            out=xt[:],
                            in_=xR[:, h2 * XCH:(h2 + 1) * XCH, cs])
                        for ktl in range(XCH):
                            kt = h2 * XCH + ktl
                            st, sp = (kt == 0), (kt == NKT - 1)
                            for h in range(QH):
                                nc.tensor.matmul(
                                    pq[h][:],
                                    wq_sb[:, kt, h * 128:(h + 1) * 128],
                                    xt[:, ktl, :], start=st, stop=sp,
                                    skip_group_check=True)
                            nc.tensor.matmul(
                                pk[:], wk_sb[:, kt, :], xt[:, ktl, :],
                                start=st, stop=sp, skip_group_check=True)
                            nc.tensor.matmul(
                                pv[:], wv_sb[:, kt, :], xt[:, ktl, :],
                                start=st, stop=sp, skip_group_check=True)

                    def rope_apply(psrc, dst_ap, tg):
                        m1 = rope.tile([128, 512], F32, name=f"m1{tg}",
                                       tag="m1")
                        nc.vector.tensor_mul(m1[:], psrc[:], cos_sb[:, cs])
                        m2 = rope.tile([128, 512], F32, name=f"m2{tg}",
                                       tag="m2")
                        nc.vector.tensor_mul(m2[0:64, :], psrc[64:128, :],
                                             ss_sb[0:64, cs])
                        nc.vector.tensor_mul(m2[64:128, :], psrc[0:64, :],
                                             ss_sb[64:128, cs])
                        nc.vector.tensor_add(dst_ap, m1[:], m2[:])

                    for h in range(QH):
                        rope_apply(pq[h], qT_sb[h][:, cs], f"_{tcn}_{h}")
                    rope_apply(pk, kT_sb[:, cs], f"k_{tcn}")
                    nc.scalar.copy(vT_sb[:, cs], pv[:])
                    # wo gamma pass interleaved (1 chunk per tcn)
                    wo_pass1(tcn)

            # wo gamma: reduce + AllReduce + scalars
            nc.vector.tensor_reduce(go1[:, 0:1], acco[:], X, ALU.add)
            nc.gpsimd.partition_all_reduce(go1r[:], go1[:], channels=128,
                                           reduce_op=RADD)
            go_sb = gacc.tile([1, 8], F32, name="go_sb")
            nc.vector.memset(go_sb[:], 0.0)
            nc.scalar.copy(go_sb[0:1, 0:1], go1r[0:1, 0:1])
            nc.sync.dma_start(out=ar2_in[:], in_=go_sb[:])
            nc.gpsimd.collective_compute(
                "AllReduce", ALU.add, replica_groups=RG,
                ins=[ar2_in[:].opt()], outs=[ar2_out[:].opt()])
            aro_sb = gacc.tile([1, 8], F32, name="aro_sb")
            nc.sync.dma_start(out=aro_sb[:], in_=ar2_out[:])
            gval2 = gacc.tile([1, 2], F32, name="gval2")
            nc.vector.tensor_scalar(gval2[0:1, 0:1], aro_sb[0:1, 0:1],
                                    1.0 / NO, 1e-5, ALU.mult, ALU.add)
            gall2 = gacc.tile([1, 2], F32, name="gall2")
            nc.vector.reciprocal(gall2[0:1, 0:1], gval2[0:1, 0:1])
            nc.vector.tensor_mul(gall2[0:1, 1:2], gval2[0:1, 0:1],
                                 gall[0:1, 5:6])
            nc.gpsimd.partition_broadcast(bsc2[:], gall2[:], channels=128)
            IO, OSC = bsc2[:, 0:1], bsc2[:, 1:2]

            xpool.release()
            wstream.release()
            wqq_p.release()
            tabp.release()

            # ---------------- attention + wo quant + o_proj -------------
            wostream = tc.alloc_tile_pool(name="wostream", bufs=1)

            def wo_quant(c):
                wc2 = wostream.tile([128, WCH, 512], F32, name=f"p2o{c}",
                                    tag="wo")
                nc.scalar.dma_start(out=wc2[:],
                                    in_=woR[:, c * WCH:(c + 1) * WCH, :])
                for i in range(WCH):
                    quant_tile(wc2[:, i, :], wo_sb[:, c * WCH + i, :], IO,
                               f"o{c}", i)

            with tc.tile_pool(name="vnatp", bufs=2) as vnatp, \
                 tc.tile_pool(name="epool", bufs=16) as epool, \
                 tc.tile_pool(name="zrp", bufs=2) as zrp, \
                 tc.tile_pool(name="zbp", bufs=2) as zbp, \
                 tc.tile_pool(name="aop", bufs=2) as aop, \
                 tc.tile_pool(name="a3", bufs=3) as a3, \
                 tc.tile_pool(name="o3", bufs=2) as o3, \
                 tc.tile_pool(name="ps_s", bufs=2, space="PSUM") as ps_s, \
                 tc.tile_pool(name="ps_po", bufs=2, space="PSUM") as ps_po, \
                 tc.tile_pool(name="ps_z", bufs=1, space="PSUM") as ps_z, \
                 tc.tile_pool(name="ps_tr", bufs=1, space="PSUM") as ps_tr, \
                 tc.tile_pool(name="p3", bufs=2, space="PSUM") as p3:

                def build_vnat(b):
                    boff = b * S
                    vnat = vnatp.tile([128, S], BF16, name=f"vnat{b}",
                                      tag="vnat")
                    for kt in range(SKT):
                        ptr = ps_tr.tile([128, 128], BF16,
                                         name=f"ptr{b}_{kt}", tag="ptr")
                        nc.tensor.transpose(
                            ptr[:],
                            vT_sb[:, boff + kt * 128:boff + (kt + 1) * 128],
                            idn[:])
                        nc.scalar.copy(vnat[:, kt * 128:(kt + 1) * 128],
                                       ptr[:])
                    return vnat

                def attn_unit(b, qc, vnat, hpair):
                    """scores for 2 heads, then softmax+AV for them."""
                    boff = b * S
                    kts = [kt for kt in range(SKT)
                           if status[b, kt, qc] != 1]
                    es = {}
                    for h in hpair:
                        qsl = qT_sb[h][:, boff + qc * 512:
                                       boff + (qc + 1) * 512]
                        for kt in kts:
                            ps_ = ps_s.tile([128, 512], F32,
                                            name=f"s{b}{h}{qc}{kt}",
                                            tag="ps")
                            nc.tensor.matmul(
                                ps_[:],
                                kT_sb[:, boff + kt * 128:
                                      boff + (kt + 1) * 128],
                                qsl, start=True, stop=True,
                                skip_group_check=True)
                            if status[b, kt, qc] == 2:
                                mi = index[(b, kt, qc)]
                                nc.vector.tensor_add(ps_[:], ps_[:],
                                                     mall[:, mi, :])
                            e = epool.tile([128, 512], BF16,
                                           name=f"e{b}{h}{qc}{kt}", tag="e")
                            nc.scalar.activation(e[:], ps_[:], ACTF.Exp,
                                                 scale=CSC)
                            es[(h, kt)] = e
                    for h in hpair:
                        pz = ps_z.tile([1, 512], F32, name=f"pz{b}{h}{qc}",
                                       tag="pz")
                        po = ps_po.tile([128, 512], F32,
                                        name=f"po{b}{h}{qc}", tag="po")
                        for i, kt in enumerate(kts):
                            fst, lst = (i == 0), (i == len(kts) - 1)
                            e = es[(h, kt)]
                            nc.tensor.matmul(pz[:], onesk[:], e[:],
                                             start=fst, stop=lst,
                                             skip_group_check=True)
                            nc.tensor.matmul(
                                po[:], vnat[:, kt * 128:(kt + 1) * 128],
                                e[:], start=fst, stop=lst,
                                skip_group_check=True)
                        zr = zrp.tile([1, 512], F32, name=f"zr{b}{h}{qc}",
                                      tag="zr")
                        nc.vector.reciprocal(zr[:], pz[:])
                        zb = zbp.tile([128, 512], F32,
                                      name=f"zb{b}{h}{qc}", tag="zb")
                        nc.gpsimd.partition_broadcast(zb[:], zr[:],
                                                      channels=128)
                        aof = aop.tile([128, 512], F32,
                                       name=f"aof{b}{h}{qc}", tag="aof")
                        nc.vector.tensor_mul(aof[:], po[:], zb[:])
                        ao = aop.tile([128, 512], BF16, name=f"ao{b}{h}{qc}",
                                      tag="ao")
                        nc.scalar.activation(ao[:], aof[:], ACTF.Copy)
                        nc.sync.dma_start(
                            out=agin[b][qc][h * 128:(h + 1) * 128, :],
                            in_=ao[:])

                def ag_chunk(b, qc):
                    nc.gpsimd.collective_compute(
                        "AllGather", ALU.bypass, replica_groups=RG,
                        ins=[agin[b][qc][:, :].opt()],
                        outs=[agout[b][qc][:, :].opt()])

                def oproj(b, qc):
                    ch = b * 2 + qc
                    ats = []
                    for hf in range(2):
                        at = a3.tile([128, NKT // 2, 512], BF16,
                                     name=f"at{ch}_{hf}", tag="at")
                        nc.scalar.dma_start(
                            out=at[:],
                            in_=agoutR[b][qc][:, hf * 16:(hf + 1) * 16, :])
                        ats.append(at)
                    for tt in range(4):
                        pout = p3.tile([128, 512], F32, name=f"po3_{ch}{tt}",
                                       tag="pout")
                        for kt in range(NKT):
                            nc.tensor.matmul(
                                pout[:],
                                ats[kt // 16][:, kt % 16,
                                              tt * 128:(tt + 1) * 128],
                                wo_sb[:, kt, :], start=(kt == 0),
                                stop=(kt == NKT - 1), skip_group_check=True)
                        osb = o3.tile([128, 512], F32, name=f"osb{ch}{tt}",
                                      tag="osb")
                        nc.scalar.activation(osb[:], pout[:], ACTF.Copy,
                                             scale=OSC)
                        nc.sync.dma_start(
                            out=outN[ch * 512 + tt * 128:
                                     ch * 512 + (tt + 1) * 128, :],
                            in_=osb[:])

                # batch 0 attention, wo quant interleaved
                vnat0 = build_vnat(0)
                attn_unit(0, 0, vnat0, (0, 1))
                wo_quant(0)
                attn_unit(0, 0, vnat0, (2, 3))
                ag_chunk(0, 0)
                wo_quant(1)
                attn_unit(0, 1, vnat0, (0, 1))
                wo_quant(2)
                attn_unit(0, 1, vnat0, (2, 3))
                ag_chunk(0, 1)
                wo_quant(3)
                # batch 1 attention with o_proj chunks interleaved
                vnat1 = build_vnat(1)
                attn_unit(1, 0, vnat1, (0, 1))
                attn_unit(1, 0, vnat1, (2, 3))
                ag_chunk(1, 0)
                oproj(0, 0)
                attn_unit(1, 1, vnat1, (0, 1))
                attn_unit(1, 1, vnat1, (2, 3))
                ag_chunk(1, 1)
                oproj(0, 1)
                oproj(1, 0)
                oproj(1, 1)

            wostream.release()
            qscr.release()
            gacc.release()
            mall_p.release()
            qkvp.release()
            wop.release()

    nc.compile()
    return nc


def kernel(hidden_states, Wq, Wk, Wv, Wo, attention_mask, position_ids):
    from concourse.bass_utils import run_bass_kernel_spmd
    from concourse.bass_interp import get_hw_module

    hs = np.ascontiguousarray(np.asarray(hidden_states, dtype=np.float32))
    Wq = np.asarray(Wq, dtype=np.float32)
    Wk = np.asarray(Wk, dtype=np.float32)
    Wv = np.asarray(Wv, dtype=np.float32)
    Wo = np.asarray(Wo, dtype=np.float32)
    mask = np.asarray(attention_mask, dtype=np.float32)
    posf = np.ascontiguousarray(
        np.asarray(position_ids).reshape(1, T).astype(np.float32))

    status, index, packed = _classify_mask(mask)
    n_packed = packed.shape[0] // 128

    key = (status.tobytes(), n_packed)
    if key not in _cache:
        nc = _build(status, index, n_packed)
        nc.m = get_hw_module(nc.m)
        _cache[key] = nc
    nc = _cache[key]

    xTb = np.ascontiguousarray(hs.reshape(T, H).T.astype(BFNP))
    in_maps = []
    for c in range(NCORES):
        in_maps.append({
            "xT": xTb,
            "wqT": np.ascontiguousarray(
                Wq[c * QH * HD:(c + 1) * QH * HD, :].T),
            "wkT": np.ascontiguousarray(Wk[c * HD:(c + 1) * HD, :].T),
            "wvT": np.ascontiguousarray(Wv[c * HD:(c + 1) * HD, :].T),
            "woT": np.ascontiguousarray(Wo[c * MSH:(c + 1) * MSH, :].T),
            "maskP": packed,
            "pos": posf,
        })
    res = run_bass_kernel_spmd(nc, in_maps, core_ids=list(range(NCORES)),
                               trace=bool(os.environ.get("BITNET_TRACE")))
    global last_exec_time_ns
    last_exec_time_ns = res.exec_time_ns
    out = np.concatenate(
        [res.results[c]["outN"] for c in range(NCORES)], axis=1)  # (T, H)
    return np.ascontiguousarray(out).reshape(B, S, H).astype(np.float32)


# revision 49
# speedup vs baseline: 1.2269x; 1.0288x over previous
"""BitNet attention block on 8 TRN2 NeuronCores (tensor-parallel over heads).

v2: bf16 datapath. Weights are stored as PURE ternary {-1,0,+1} bf16; all
absmean gammas are folded into three scalar application points (the exp's
input scale c = gq*gk/sqrt(HD), the mask pre-scale 1/c, and the o_proj
output scale gv*go). Quantization streams the f32 weights twice from HBM
(sum pass for gamma, quant pass) through a small rotating SBUF buffer, so
phase-1 matmuls start as soon as the first weight chunk is quantized.
DMAs are issued as single multi-tile instructions over rearranged
[128, kt, col] views so descriptors spread across all 16 DMA engines.
Partition reductions/broadcasts run on the (otherwise idle) GpSimd engine.
AllGather is chunked per (batch, half) in bf16 and o_proj chunks are
interleaved with the second batch's attention.

Sharding: core c owns Q heads [4c,4c+4), KV head c, o_proj output dims
[512c, 512c+512). Host does layout transforms (transpose/shard/cast) only.
"""
import os
import sys
sys.path.insert(0, "/opt/trn_rl_repo")
import numpy as np
import ml_dtypes

BFNP = ml_dtypes.bfloat16
B, S, H = 2, 1024, 4096
NH, NKV, HD = 32, 8, 128
NCORES = 8
T = B * S
QH = NH // NCORES          # 4 q-heads per core
MSH = H // NCORES          # 512 o_proj out-dims per core
THETA = 10000.0
C_MAGIC = 12582912.0       # 1.5 * 2**23: (x + C) - C == round-half-even(x)
TWO_PI = 6.283185307179586
NKT = H // 128             # 32 contraction tiles
NTC = T // 512             # 4 token chunks
SKT = S // 128             # 8 score k-tiles per batch
SQC = S // 512             # 2 q-chunks per batch
WCH = 4                    # kt per wq/wo quant chunk (8 chunks each)
XCH = 8                    # kt per x chunk (4 chunks per tcn)

NQ = float(NH * HD * H)
NK = float(NKV * HD * H)
NO = float(H * NH * HD)

_cache = {}
last_exec_time_ns = None


def _classify_mask(mask):
    """Per (b, kt, qc) [128k x 512q] block: 0 zero, 1 masked-out, 2 general."""
    status = np.empty((B, SKT, SQC), dtype=np.int8)
    index = {}
    packed = []
    for b in range(B):
        mb = np.asarray(mask[b, 0], dtype=np.float32)   # (q, k)
        for kt in range(SKT):
            for qc in range(SQC):
                blk = mb[qc * 512:(qc + 1) * 512, kt * 128:(kt + 1) * 128]
                if not blk.any():
                    status[b, kt, qc] = 0
                elif (blk <= -1e4).all():
                    status[b, kt, qc] = 1
                else:
                    status[b, kt, qc] = 2
                    index[(b, kt, qc)] = len(packed)
                    packed.append(np.ascontiguousarray(blk.T))  # (128k, 512q)
    if packed:
        packed_arr = np.concatenate(packed, axis=0)
    else:
        packed_arr = np.zeros((128, 512), dtype=np.float32)
    return status, index, packed_arr.astype(BFNP)


def _cody_consts():
    c1 = float(np.float32(6.28125))
    r = np.float64(TWO_PI) - c1
    c2 = float(np.float32(r - np.remainder(r, 2.0 ** -24)))
    c3 = float(np.float32(np.float64(TWO_PI) - c1 - float(c2)))
    return c1, c2, c3


def _build(status, index, n_packed):
    from concourse import bacc, tile, mybir, bass_isa

    F32 = mybir.dt.float32
    BF16 = mybir.dt.bfloat16
    ACTF = mybir.ActivationFunctionType
    ALU = mybir.AluOpType
    X = mybir.AxisListType.X
    RADD = bass_isa.ReduceOp.add
    RG = [list(range(NCORES))]
    c1, c2, c3 = _cody_consts()

    nc = bacc.Bacc("TRN2", target_bir_lowering=False, debug=False,
                   num_devices=NCORES)

    xT = nc.dram_tensor("xT", [H, T], BF16, kind="ExternalInput")
    wqT = nc.dram_tensor("wqT", [H, QH * HD], F32, kind="ExternalInput")
    wkT = nc.dram_tensor("wkT", [H, HD], F32, kind="ExternalInput")
    wvT = nc.dram_tensor("wvT", [H, HD], F32, kind="ExternalInput")
    woT = nc.dram_tensor("woT", [H, MSH], F32, kind="ExternalInput")
    maskP = nc.dram_tensor("maskP", [n_packed * 128, 512], BF16,
                           kind="ExternalInput")
    pos = nc.dram_tensor("pos", [1, T], F32, kind="ExternalInput")
    outN = nc.dram_tensor("outN", [T, MSH], F32, kind="ExternalOutput")

    # [128, kt, col] views: element (p, k, c) = tensor[128k + p, c]
    wqR = wqT[:, :].rearrange("(k p) c -> p k c", p=128)
    wkR = wkT[:, :].rearrange("(k p) c -> p k c", p=128)
    wvR = wvT[:, :].rearrange("(k p) c -> p k c", p=128)
    woR = woT[:, :].rearrange("(k p) c -> p k c", p=128)
    xR = xT[:, :].rearrange("(k p) c -> p k c", p=128)
    mR = maskP[:, :].rearrange("(k p) c -> p k c", p=128)

    idn_c = nc.inline_tensor(np.eye(128, dtype=np.float32), name="idn_c")
    invf_np = (1.0 / THETA ** (np.arange(0, HD, 2, dtype=np.float32) / HD))
    invf_np = np.concatenate([invf_np, invf_np]).reshape(HD, 1)
    invf_c = nc.inline_tensor(invf_np.astype(np.float32), name="invf_c")

    with tile.TileContext(nc) as tc, \
         nc.allow_low_precision(reason="bf16 kernel"):
        with tc.tile_pool(name="cpool", bufs=1) as cpool, \
             tc.tile_pool(name="dbounce", bufs=1, space="DRAM") as dbounce:
            # DRAM bounce tiles (tracked by Tile for collective deps)
            ar_in = dbounce.tile([1, 8], F32, name="ar_in")
            ar_out = dbounce.tile([1, 8], F32, name="ar_out",
                                  addr_space="Shared")
            ar2_in = dbounce.tile([1, 8], F32, name="ar2_in")
            ar2_out = dbounce.tile([1, 8], F32, name="ar2_out",
                                   addr_space="Shared")
            agin = [[dbounce.tile([QH * HD, 512], BF16, name=f"agin{b}_{qc}")
                     for qc in range(SQC)] for b in range(B)]
            agout = [[dbounce.tile([H, 512], BF16, name=f"agout{b}_{qc}",
                                   addr_space="Shared") for qc in range(SQC)]
                     for b in range(B)]
            agoutR = [[agout[b][qc][:, :].rearrange("(k p) c -> p k c", p=128)
                       for qc in range(SQC)] for b in range(B)]

            idn_f = cpool.tile([128, 128], F32, name="idn_f")
            nc.sync.dma_start(out=idn_f[:], in_=idn_c[:, :])
            idn = cpool.tile([128, 128], BF16, name="idn")
            nc.scalar.copy(idn[:], idn_f[:])
            onesk = cpool.tile([128, 1], BF16, name="onesk")
            nc.vector.memset(onesk[:], 1.0)
            invf = cpool.tile([128, 1], F32, name="invf")
            nc.sync.dma_start(out=invf[:], in_=invf_c[:, :])
            cmag = cpool.tile([128, 1], F32, name="cmag")
            nc.vector.memset(cmag[:], C_MAGIC)
            cmag1 = cpool.tile([128, 1], F32, name="cmag1")
            nc.vector.memset(cmag1[:], C_MAGIC + 1.0)
            # broadcast scalar columns:
            #   bsc: 0=1/gq 1=1/gk 2=1/gv 3=c 4=1/c
            #   bsc2: 0=1/go 1=gv*go
            bsc = cpool.tile([128, 8], F32, name="bsc")
            bsc2 = cpool.tile([128, 2], F32, name="bsc2")

            # ---------------- pool allocation (LIFO by release time) ----
            # bottom: live until the end
            wop = tc.alloc_tile_pool(name="wop", bufs=1)
            wo_sb = wop.tile([128, NKT, MSH], BF16, name="wo_sb")
            qkvp = tc.alloc_tile_pool(name="qkvp", bufs=1)
            qT_sb = [qkvp.tile([128, T], BF16, name=f"qT{h}")
                     for h in range(QH)]
            kT_sb = qkvp.tile([128, T], BF16, name="kT_sb")
            vT_sb = qkvp.tile([128, T], BF16, name="vT_sb")
            gacc = tc.alloc_tile_pool(name="gacc", bufs=1)
            qscr = tc.alloc_tile_pool(name="qscr", bufs=3)
            # released at end of phase 1 (LIFO: xpool, wstream, wqq_p, tabp)
            tabp = tc.alloc_tile_pool(name="tabp", bufs=1)
            cos_sb = tabp.tile([128, T], BF16, name="cos_sb")
            ss_sb = tabp.tile([128, T], BF16, name="ss_sb")

            # ---------------- RoPE tables (scratch freed before weights)
            with tc.tile_pool(name="rtab", bufs=2) as rtab:
                for tcn in range(NTC):
                    cs = slice(tcn * 512, (tcn + 1) * 512)
                    pchunk = rtab.tile([1, 512], F32, name=f"pos{tcn}",
                                       tag="pos")
                    nc.sync.dma_start(out=pchunk[:], in_=pos[0:1, cs])
                    pf = rtab.tile([128, 512], F32, name=f"pf{tcn}", tag="pf")
                    nc.gpsimd.partition_broadcast(pf[:], pchunk[:],
                                                  channels=128)
                    f_sb = rtab.tile([128, 512], F32, name=f"f{tcn}", tag="f")
                    nc.scalar.activation(f_sb[:], pf[:], ACTF.Copy,
                                         scale=invf[:])
                    k_sb = rtab.tile([128, 512], F32, name=f"kk{tcn}",
                                     tag="kk")
                    nc.vector.tensor_scalar(k_sb[:], f_sb[:], 1.0 / TWO_PI,
                                            C_MAGIC, ALU.mult, ALU.add)
                    nc.vector.tensor_scalar(k_sb[:], k_sb[:], C_MAGIC, None,
                                            ALU.subtract)
                    y_sb = rtab.tile([128, 512], F32, name=f"y{tcn}", tag="y")
                    nc.vector.scalar_tensor_tensor(
                        y_sb[:], k_sb[:], -c1, f_sb[:], ALU.mult, ALU.add)
                    nc.vector.scalar_tensor_tensor(
                        y_sb[:], k_sb[:], -c2, y_sb[:], ALU.mult, ALU.add)
                    nc.vector.scalar_tensor_tensor(
                        y_sb[:], k_sb[:], -c3, y_sb[:], ALU.mult, ALU.add)
                    nc.scalar.activation(ss_sb[0:64, cs], y_sb[0:64, :],
                                         ACTF.Sin, scale=-1.0)
                    nc.scalar.activation(ss_sb[64:128, cs], y_sb[64:128, :],
                                         ACTF.Sin)
                    yc = rtab.tile([128, 512], F32, name=f"yc{tcn}", tag="yc")
                    nc.vector.tensor_scalar(yc[:], y_sb[:],
                                            float(np.pi / 2), None, ALU.add)
                    m_sb = rtab.tile([128, 512], F32, name=f"mm{tcn}",
                                     tag="mm")
                    nc.vector.tensor_scalar(m_sb[:], yc[:], float(np.pi),
                                            None, ALU.is_gt)
                    nc.vector.scalar_tensor_tensor(
                        yc[:], m_sb[:], -TWO_PI, yc[:], ALU.mult, ALU.add)
                    nc.scalar.activation(cos_sb[:, cs], yc[:], ACTF.Sin)

            wqq_p = tc.alloc_tile_pool(name="wqq_p", bufs=1)
            wq_sb = wqq_p.tile([128, NKT, QH * HD], BF16, name="wq_sb")
            wk_sb = wqq_p.tile([128, NKT, HD], BF16, name="wk_sb")
            wv_sb = wqq_p.tile([128, NKT, HD], BF16, name="wv_sb")
            wkvp = tc.alloc_tile_pool(name="wkvp", bufs=1)
            wckv = wkvp.tile([128, NKT, 2 * HD], F32, name="p2kv")
            wstream = tc.alloc_tile_pool(name="wstream", bufs=2)
            xpool = tc.alloc_tile_pool(name="xpool", bufs=2)

            # ---------------- weight pass 1: |w| sums -------------------
            accq = gacc.tile([128, NKT], F32, name="accq")
            acck = gacc.tile([128, NKT], F32, name="acck")
            accv = gacc.tile([128, NKT], F32, name="accv")
            acco = gacc.tile([128, NKT], F32, name="acco")
            g4 = gacc.tile([128, 4], F32, name="g4")
            g4r = gacc.tile([128, 4], F32, name="g4r")
            go1 = gacc.tile([128, 1], F32, name="go1")
            go1r = gacc.tile([128, 1], F32, name="go1r")

            NWCH = NKT // WCH
            for c in range(NWCH):
                wc = wstream.tile([128, WCH, 512], F32, name=f"p1q{c}",
                                  tag="wq")
                nc.sync.dma_start(out=wc[:],
                                  in_=wqR[:, c * WCH:(c + 1) * WCH, :])
                for i in range(WCH):
                    nc.vector.tensor_reduce(
                        accq[:, c * WCH + i:c * WCH + i + 1], wc[:, i, :], X,
                        ALU.add, apply_absolute_value=True)
            for hf in range(2):
                k0 = hf * (NKT // 2)
                wck = wstream.tile([128, NKT // 2, HD], F32, name=f"p1k{hf}",
                                   tag="wq")
                nc.sync.dma_start(out=wck[:],
                                  in_=wkR[:, k0:k0 + NKT // 2, :])
                for i in range(NKT // 2):
                    nc.vector.tensor_reduce(acck[:, k0 + i:k0 + i + 1],
                                            wck[:, i, :], X, ALU.add,
                                            apply_absolute_value=True)
                wcv = wstream.tile([128, NKT // 2, HD], F32, name=f"p1v{hf}",
                                   tag="wq")
                nc.sync.dma_start(out=wcv[:],
                                  in_=wvR[:, k0:k0 + NKT // 2, :])
                for i in range(NKT // 2):
                    nc.vector.tensor_reduce(accv[:, k0 + i:k0 + i + 1],
                                            wcv[:, i, :], X, ALU.add,
                                            apply_absolute_value=True)
            nc.vector.tensor_reduce(g4[:, 0:1], accq[:], X, ALU.add)
            nc.vector.tensor_reduce(g4[:, 1:2], acck[:], X, ALU.add)
            nc.vector.tensor_reduce(g4[:, 2:3], accv[:], X, ALU.add)
            nc.vector.memset(g4[:, 3:4], 0.0)
            nc.gpsimd.partition_all_reduce(g4r[:], g4[:], channels=128,
                                           reduce_op=RADD)
            gq_sb = gacc.tile([1, 8], F32, name="gq_sb")
            nc.vector.memset(gq_sb[:], 0.0)
            nc.scalar.copy(gq_sb[0:1, 0:4], g4r[0:1, 0:4])
            nc.sync.dma_start(out=ar_in[:], in_=gq_sb[:])
            nc.gpsimd.collective_compute(
                "AllReduce", ALU.add, replica_groups=RG,
                ins=[ar_in[:].opt()], outs=[ar_out[:].opt()])
            arq_sb = gacc.tile([1, 8], F32, name="arq_sb")
            # readback on the DVE queue so the in-order sync queue can move
            # on to the pass-2 weight and x-prefetch DMAs (which are thereby
            # gated behind ar_in's wait => pass 1 gets full DMA bandwidth)
            nc.scalar.dma_start(out=arq_sb[:], in_=ar_out[:])

            # prefetch first x chunks (land right after pass-2 starts)
            xts0 = []
            for h2 in range(2):
                xt = xpool.tile([128, XCH, 512], BF16, name=f"x0_{h2}",
                                tag="xt")
                nc.sync.dma_start(out=xt[:],
                                  in_=xR[:, h2 * XCH:(h2 + 1) * XCH, 0:512])
                xts0.append(xt)

            # gamma math on partition 0 lanes
            gval = gacc.tile([1, 4], F32, name="gval")
            nc.vector.tensor_scalar(gval[0:1, 0:1], arq_sb[0:1, 0:1],
                                    1.0 / NQ, 1e-5, ALU.mult, ALU.add)
            nc.vector.tensor_scalar(gval[0:1, 1:3], arq_sb[0:1, 1:3],
                                    1.0 / NK, 1e-5, ALU.mult, ALU.add)
            gall = gacc.tile([1, 8], F32, name="gall")
            nc.vector.memset(gall[:], 0.0)
            nc.vector.reciprocal(gall[0:1, 0:3], gval[0:1, 0:3])
            nc.vector.tensor_mul(gall[0:1, 3:4], gval[0:1, 0:1],
                                 gval[0:1, 1:2])
            nc.vector.tensor_scalar(gall[0:1, 3:4], gall[0:1, 3:4],
                                    float(1.0 / np.sqrt(HD)), None, ALU.mult)
            nc.vector.reciprocal(gall[0:1, 4:5], gall[0:1, 3:4])
            nc.scalar.copy(gall[0:1, 5:6], gval[0:1, 2:3])
            nc.gpsimd.partition_broadcast(bsc[:], gall[:], channels=128)
            IQ, IK, IV = (bsc[:, i:i + 1] for i in range(3))
            CSC, ICS = bsc[:, 3:4], bsc[:, 4:5]

            # ---------------- quantized weights (pure ternary bf16) -----
            def quant_tile(src, dst, inv_ap, tagn, i, fsz):
                """dst_bf16 = clip(round_half_even(src/gamma), -1, 1).

                Round via the +C magic store (ACT); clip in magic space via
                two single-op DVE max/min against C-1/C+1; the final ACT
                Copy subtracts C exactly and converts to bf16. Avoids the
                slow DVE paths (dual-op ALU, bf16 output).
                """
                t = qscr.tile(list(src.shape), F32, name=f"qt{tagn}_{i}",
                              tag=f"qa{fsz}")
                nc.scalar.activation(t[:], src, ACTF.Identity,
                                     bias=cmag[:], scale=inv_ap)
                nc.vector.tensor_scalar(t[:], t[:], C_MAGIC - 1.0, None,
                                        ALU.max)
                nc.vector.tensor_scalar(t[:], t[:], C_MAGIC + 1.0, None,
                                        ALU.min)
                nc.scalar.activation(dst, t[:], ACTF.Copy, bias=-C_MAGIC)

            # pass-2: persistent k/v staging (one tile, two slice DMAs) and
            # 4-kt wq chunks, with quant interleaved in PE consumption order
            nc.sync.dma_start(out=wckv[:, :, 0:HD], in_=wkR[:, :, :])
            nc.sync.dma_start(out=wckv[:, :, HD:2 * HD], in_=wvR[:, :, :])
            for c in range(NWCH):
                wc2 = wstream.tile([128, WCH, 512], F32, name=f"p2q{c}",
                                   tag="wq")
                nc.sync.dma_start(out=wc2[:],
                                  in_=wqR[:, c * WCH:(c + 1) * WCH, :])
                for i in range(WCH):
                    quant_tile(wc2[:, i, :], wq_sb[:, c * WCH + i, :], IQ,
                               f"q{c}", i, 512)
                for p in (2 * c, 2 * c + 1):
                    quant_tile(wckv[:, 2 * p:2 * p + 2, 0:HD],
                               wk_sb[:, 2 * p:2 * p + 2, :], IK,
                               f"k{c}", p, 256)
                    quant_tile(wckv[:, 2 * p:2 * p + 2, HD:2 * HD],
                               wv_sb[:, 2 * p:2 * p + 2, :], IV,
                               f"v{c}", p, 256)

            # ---------------- phase 1: QKV projections + RoPE -----------
            NXC = NKT // XCH

            # wo pass 1 tiles interleaved into phase 1 below
            def wo_pass1(c):
                wco = wstream.tile([128, WCH, 512], F32, name=f"p1o{c}",
                                   tag="wq")
                nc.sync.dma_start(out=wco[:],
                                  in_=woR[:, c * WCH:(c + 1) * WCH, :])
                for i in range(WCH):
                    nc.vector.tensor_reduce(
                        acco[:, c * WCH + i:c * WCH + i + 1], wco[:, i, :],
                        X, ALU.add, apply_absolute_value=True)

            with tc.tile_pool(name="rope", bufs=2) as rope, \
                 tc.tile_pool(name="p1", bufs=8, space="PSUM") as p1:
                for tcn in range(NTC):
                    cs = slice(tcn * 512, (tcn + 1) * 512)
                    pq = [p1.tile([128, 512], F32, name=f"pq{tcn}_{h}",
                                  tag="p1") for h in range(QH)]
                    pk = p1.tile([128, 512], F32, name=f"pk{tcn}", tag="p1")
                    pv = p1.tile([128, 512], F32, name=f"pv{tcn}", tag="p1")
                    for h2 in range(NXC):
                        if tcn == 0 and h2 < 2:
                            xt = xts0[h2]
                        else:
                            xt = xpool.tile([128, XCH, 512], BF16,
                                            name=f"x{tcn}_{h2}", tag="xt")
                            nc.sync.dma_start(
                                out=xt[:],
                                in_=xR[:, h2 * XCH:(h2 + 1) * XCH, cs])
                        for ktl in range(XCH):
                            kt = h2 * XCH + ktl
                            st, sp = (kt == 0), (kt == NKT - 1)
                            for h in range(QH):
                                nc.tensor.matmul(
                                    pq[h][:],
                                    wq_sb[:, kt, h * 128:(h + 1) * 128],
                                    xt[:, ktl, :], start=st, stop=sp,
                                    skip_group_check=True)
                            nc.tensor.matmul(
                                pk[:], wk_sb[:, kt, :], xt[:, ktl, :],
                                start=st, stop=sp, skip_group_check=True)
                            nc.tensor.matmul(
                                pv[:], wv_sb[:, kt, :], xt[:, ktl, :],
                                start=st, stop=sp, skip_group_check=True)

                    def rope_apply(psrc, dst_ap, tg):
                        m1 = rope.tile([128, 512], F32, name=f"m1{tg}",
                                       tag="m1")
                        nc.vector.tensor_mul(m1[:], psrc[:], cos_sb[:, cs])
                        m2 = rope.tile([128, 512], F32, name=f"m2{tg}",
                                       tag="m2")
                        nc.vector.tensor_mul(m2[0:64, :], psrc[64:128, :],
                                             ss_sb[0:64, cs])
                        nc.vector.tensor_mul(m2[64:128, :], psrc[0:64, :],
                                             ss_sb[64:128, cs])
                        nc.vector.tensor_add(dst_ap, m1[:], m2[:])

                    for h in range(QH):
                        rope_apply(pq[h], qT_sb[h][:, cs], f"_{tcn}_{h}")
                    rope_apply(pk, kT_sb[:, cs], f"k_{tcn}")
                    nc.scalar.copy(vT_sb[:, cs], pv[:])
                    # wo gamma pass interleaved (2 chunks per tcn)
                    wo_pass1(2 * tcn)
                    wo_pass1(2 * tcn + 1)

            # wo gamma: reduce + AllReduce + scalars
            nc.vector.tensor_reduce(go1[:, 0:1], acco[:], X, ALU.add)
            nc.gpsimd.partition_all_reduce(go1r[:], go1[:], channels=128,
                                           reduce_op=RADD)
            go_sb = gacc.tile([1, 8], F32, name="go_sb")
            nc.vector.memset(go_sb[:], 0.0)
            nc.scalar.copy(go_sb[0:1, 0:1], go1r[0:1, 0:1])
            nc.sync.dma_start(out=ar2_in[:], in_=go_sb[:])
            nc.gpsimd.collective_compute(
                "AllReduce", ALU.add, replica_groups=RG,
                ins=[ar2_in[:].opt()], outs=[ar2_out[:].opt()])
            aro_sb = gacc.tile([1, 8], F32, name="aro_sb")
            nc.gpsimd.dma_start(out=aro_sb[:], in_=ar2_out[:])
            gval2 = gacc.tile([1, 2], F32, name="gval2")
            nc.vector.tensor_scalar(gval2[0:1, 0:1], aro_sb[0:1, 0:1],
                                    1.0 / NO, 1e-5, ALU.mult, ALU.add)
            gall2 = gacc.tile([1, 2], F32, name="gall2")
            nc.vector.reciprocal(gall2[0:1, 0:1], gval2[0:1, 0:1])
            nc.vector.tensor_mul(gall2[0:1, 1:2], gval2[0:1, 0:1],
                                 gall[0:1, 5:6])
            nc.gpsimd.partition_broadcast(bsc2[:], gall2[:], channels=128)
            IO, OSC = bsc2[:, 0:1], bsc2[:, 1:2]

            xpool.release()
            wstream.release()
            wkvp.release()
            wqq_p.release()
            tabp.release()

            # ---------------- masks (scaled by 1/c) ---------------------
            mall_p = tc.alloc_tile_pool(name="mall_p", bufs=1)
            mall = mall_p.tile([128, n_packed, 512], BF16, name="mall")
            nc.gpsimd.dma_start(out=mall[:], in_=mR[:, :, :])
            for mi in range(n_packed):
                nc.scalar.activation(mall[:, mi, :], mall[:, mi, :],
                                     ACTF.Copy, scale=ICS)

            # ---------------- attention + wo quant + o_proj -------------
            wostream = tc.alloc_tile_pool(name="wostream", bufs=1)

            def wo_quant(c):
                wc2 = wostream.tile([128, WCH, 512], F32, name=f"p2o{c}",
                                    tag="wo")
                nc.scalar.dma_start(out=wc2[:],
                                    in_=woR[:, c * WCH:(c + 1) * WCH, :])
                for i in range(WCH):
                    quant_tile(wc2[:, i, :], wo_sb[:, c * WCH + i, :], IO,
                               f"o{c}", i, 512)

            with tc.tile_pool(name="vnatp", bufs=2) as vnatp, \
                 tc.tile_pool(name="epool", bufs=16) as epool, \
                 tc.tile_pool(name="zrp", bufs=2) as zrp, \
                 tc.tile_pool(name="zbp", bufs=2) as zbp, \
                 tc.tile_pool(name="aop", bufs=2) as aop, \
                 tc.tile_pool(name="a3", bufs=3) as a3, \
                 tc.tile_pool(name="o3", bufs=2) as o3, \
                 tc.tile_pool(name="ps_s", bufs=2, space="PSUM") as ps_s, \
                 tc.tile_pool(name="ps_po", bufs=2, space="PSUM") as ps_po, \
                 tc.tile_pool(name="ps_z", bufs=1, space="PSUM") as ps_z, \
                 tc.tile_pool(name="ps_tr", bufs=1, space="PSUM") as ps_tr, \
                 tc.tile_pool(name="p3", bufs=2, space="PSUM") as p3:

                def build_vnat(b):
                    boff = b * S
                    vnat = vnatp.tile([128, S], BF16, name=f"vnat{b}",
                                      tag="vnat")
                    for kt in range(SKT):
                        ptr = ps_tr.tile([128, 128], BF16,
                                         name=f"ptr{b}_{kt}", tag="ptr")
                        nc.tensor.transpose(
                            ptr[:],
                            vT_sb[:, boff + kt * 128:boff + (kt + 1) * 128],
                            idn[:])
                        nc.scalar.copy(vnat[:, kt * 128:(kt + 1) * 128],
                                       ptr[:])
                    return vnat

                def attn_unit(b, qc, vnat, hpair):
                    """scores for 2 heads, then softmax+AV for them."""
                    boff = b * S
                    kts = [kt for kt in range(SKT)
                           if status[b, kt, qc] != 1]
                    es = {}
                    for h in hpair:
                        qsl = qT_sb[h][:, boff + qc * 512:
                                       boff + (qc + 1) * 512]
                        for kt in kts:
                            ps_ = ps_s.tile([128, 512], F32,
                                            name=f"s{b}{h}{qc}{kt}",
                                            tag="ps")
                            nc.tensor.matmul(
                                ps_[:],
                                kT_sb[:, boff + kt * 128:
                                      boff + (kt + 1) * 128],
                                qsl, start=True, stop=True,
                                skip_group_check=True)
                            if status[b, kt, qc] == 2:
                                mi = index[(b, kt, qc)]
                                nc.vector.tensor_add(ps_[:], ps_[:],
                                                     mall[:, mi, :])
                            e = epool.tile([128, 512], BF16,
                                           name=f"e{b}{h}{qc}{kt}", tag="e")
                            nc.scalar.activation(e[:], ps_[:], ACTF.Exp,
                                                 scale=CSC)
                            es[(h, kt)] = e
                    for h in hpair:
                        pz = ps_z.tile([1, 512], F32, name=f"pz{b}{h}{qc}",
                                       tag="pz")
                        po = ps_po.tile([128, 512], F32,
                                        name=f"po{b}{h}{qc}", tag="po")
                        for i, kt in enumerate(kts):
                            fst, lst = (i == 0), (i == len(kts) - 1)
                            e = es[(h, kt)]
                            nc.tensor.matmul(pz[:], onesk[:], e[:],
                                             start=fst, stop=lst,
                                             skip_group_check=True)
                            nc.tensor.matmul(
                                po[:], vnat[:, kt * 128:(kt + 1) * 128],
                                e[:], start=fst, stop=lst,
                                skip_group_check=True)
                        zr = zrp.tile([1, 512], F32, name=f"zr{b}{h}{qc}",
                                      tag="zr")
                        nc.vector.reciprocal(zr[:], pz[:])
                        zb = zbp.tile([128, 512], F32,
                                      name=f"zb{b}{h}{qc}", tag="zb")
                        nc.gpsimd.partition_broadcast(zb[:], zr[:],
                                                      channels=128)
                        aof = aop.tile([128, 512], F32,
                                       name=f"aof{b}{h}{qc}", tag="aof")
                        nc.vector.tensor_mul(aof[:], po[:], zb[:])
                        ao = aop.tile([128, 512], BF16, name=f"ao{b}{h}{qc}",
                                      tag="ao")
                        nc.scalar.activation(ao[:], aof[:], ACTF.Copy)
                        nc.sync.dma_start(
                            out=agin[b][qc][h * 128:(h + 1) * 128, :],
                            in_=ao[:])

                def ag_chunk(b, qc):
                    nc.gpsimd.collective_compute(
                        "AllGather", ALU.bypass, replica_groups=RG,
                        ins=[agin[b][qc][:, :].opt()],
                        outs=[agout[b][qc][:, :].opt()])

                def oproj(b, qc):
                    ch = b * 2 + qc
                    ats = []
                    for hf in range(2):
                        at = a3.tile([128, NKT // 2, 512], BF16,
                                     name=f"at{ch}_{hf}", tag="at")
                        nc.scalar.dma_start(
                            out=at[:],
                            in_=agoutR[b][qc][:, hf * 16:(hf + 1) * 16, :])
                        ats.append(at)
                    for tt in range(4):
                        pout = p3.tile([128, 512], F32, name=f"po3_{ch}{tt}",
                                       tag="pout")
                        for kt in range(NKT):
                            nc.tensor.matmul(
                                pout[:],
                                ats[kt // 16][:, kt % 16,
                                              tt * 128:(tt + 1) * 128],
                                wo_sb[:, kt, :], start=(kt == 0),
                                stop=(kt == NKT - 1), skip_group_check=True)
                        osb = o3.tile([128, 512], F32, name=f"osb{ch}{tt}",
                                      tag="osb")
                        nc.scalar.activation(osb[:], pout[:], ACTF.Copy,
                                             scale=OSC)
                        nc.sync.dma_start(
                            out=outN[ch * 512 + tt * 128:
                                     ch * 512 + (tt + 1) * 128, :],
                            in_=osb[:])

                # batch 0 attention, wo quant interleaved
                vnat0 = build_vnat(0)
                attn_unit(0, 0, vnat0, (0, 1))
                wo_quant(0)
                wo_quant(1)
                attn_unit(0, 0, vnat0, (2, 3))
                ag_chunk(0, 0)
                wo_quant(2)
                wo_quant(3)
                attn_unit(0, 1, vnat0, (0, 1))
                wo_quant(4)
                wo_quant(5)
                attn_unit(0, 1, vnat0, (2, 3))
                ag_chunk(0, 1)
                wo_quant(6)
                wo_quant(7)
                # batch 1 attention with o_proj chunks interleaved
                vnat1 = build_vnat(1)
                attn_unit(1, 0, vnat1, (0, 1))
                attn_unit(1, 0, vnat1, (2, 3))
                ag_chunk(1, 0)
                oproj(0, 0)
                attn_unit(1, 1, vnat1, (0, 1))
                attn_unit(1, 1, vnat1, (2, 3))
                ag_chunk(1, 1)
                oproj(0, 1)
                oproj(1, 0)
                oproj(1, 1)

            wostream.release()
            mall_p.release()
            qscr.release()
            gacc.release()
            qkvp.release()
            wop.release()

    nc.compile()
    return nc


def kernel(hidden_states, Wq, Wk, Wv, Wo, attention_mask, position_ids):
    from concourse.bass_utils import run_bass_kernel_spmd
    from concourse.bass_interp import get_hw_module

    hs = np.ascontiguousarray(np.asarray(hidden_states, dtype=np.float32))
    Wq = np.asarray(Wq, dtype=np.float32)
    Wk = np.asarray(Wk, dtype=np.float32)
    Wv = np.asarray(Wv, dtype=np.float32)
    Wo = np.asarray(Wo, dtype=np.float32)
    mask = np.asarray(attention_mask, dtype=np.float32)
    posf = np.ascontiguousarray(
        np.asarray(position_ids).reshape(1, T).astype(np.float32))

    status, index, packed = _classify_mask(mask)
    n_packed = packed.shape[0] // 128

    key = (status.tobytes(), n_packed)
    if key not in _cache:
        nc = _build(status, index, n_packed)
        nc.m = get_hw_module(nc.m)
        _cache[key] = nc
    nc = _cache[key]

    xTb = np.ascontiguousarray(hs.reshape(T, H).T.astype(BFNP))
    in_maps = []
    for c in range(NCORES):
        in_maps.append({
            "xT": xTb,
            "wqT": np.ascontiguousarray(
                Wq[c * QH * HD:(c + 1) * QH * HD, :].T),
            "wkT": np.ascontiguousarray(Wk[c * HD:(c + 1) * HD, :].T),
            "wvT": np.ascontiguousarray(Wv[c * HD:(c + 1) * HD, :].T),
            "woT": np.ascontiguousarray(Wo[c * MSH:(c + 1) * MSH, :].T),
            "maskP": packed,
            "pos": posf,
        })
    res = run_bass_kernel_spmd(nc, in_maps, core_ids=list(range(NCORES)),
                               trace=bool(os.environ.get("BITNET_TRACE")))
    global last_exec_time_ns
    last_exec_time_ns = res.exec_time_ns
    out = np.concatenate(
        [res.results[c]["outN"] for c in range(NCORES)], axis=1)  # (T, H)
    return np.ascontiguousarray(out).reshape(B, S, H).astype(np.float32)


# revision 60
# speedup vs baseline: 1.3234x; 1.0786x over previous
"""BitNet attention block on 8 TRN2 NeuronCores (tensor-parallel over heads).

v2: bf16 datapath. Weights are stored as PURE ternary {-1,0,+1} bf16; all
absmean gammas are folded into three scalar application points (the exp's
input scale c = gq*gk/sqrt(HD), the mask pre-scale 1/c, and the o_proj
output scale gv*go). Quantization streams the f32 weights twice from HBM
(sum pass for gamma, quant pass) through a small rotating SBUF buffer, so
phase-1 matmuls start as soon as the first weight chunk is quantized.
DMAs are issued as single multi-tile instructions over rearranged
[128, kt, col] views so descriptors spread across all 16 DMA engines.
Partition reductions/broadcasts run on the (otherwise idle) GpSimd engine.
AllGather is chunked per (batch, half) in bf16 and o_proj chunks are
interleaved with the second batch's attention.

Sharding: core c owns Q heads [4c,4c+4), KV head c, o_proj output dims
[512c, 512c+512). Host does layout transforms (transpose/shard/cast) only.
"""
import os
import sys
sys.path.insert(0, "/opt/trn_rl_repo")
import numpy as np
import ml_dtypes

BFNP = ml_dtypes.bfloat16
B, S, H = 2, 1024, 4096
NH, NKV, HD = 32, 8, 128
NCORES = 8
T = B * S
QH = NH // NCORES          # 4 q-heads per core
MSH = H // NCORES          # 512 o_proj out-dims per core
THETA = 10000.0
C_MAGIC = 12582912.0       # 1.5 * 2**23: (x + C) - C == round-half-even(x)
TWO_PI = 6.283185307179586
NKT = H // 128             # 32 contraction tiles
NTC = T // 512             # 4 token chunks
SKT = S // 128             # 8 score k-tiles per batch
SQC = S // 512             # 2 q-chunks per batch
WCH = 4                    # kt per wq/wo quant chunk (8 chunks each)
XCH = 8                    # kt per x chunk (4 chunks per tcn)

NQ = float(NH * HD * H)
NK = float(NKV * HD * H)
NO = float(H * NH * HD)

_cache = {}
last_exec_time_ns = None


def _classify_mask(mask):
    """Per (b, kt, qc) [128k x 512q] block: 0 zero, 1 masked-out, 2 general."""
    status = np.empty((B, SKT, SQC), dtype=np.int8)
    index = {}
    packed = []
    for b in range(B):
        mb = np.asarray(mask[b, 0], dtype=np.float32)   # (q, k)
        for kt in range(SKT):
            for qc in range(SQC):
                blk = mb[qc * 512:(qc + 1) * 512, kt * 128:(kt + 1) * 128]
                if not blk.any():
                    status[b, kt, qc] = 0
                elif (blk <= -1e4).all():
                    status[b, kt, qc] = 1
                else:
                    status[b, kt, qc] = 2
                    index[(b, kt, qc)] = len(packed)
                    packed.append(np.ascontiguousarray(blk.T))  # (128k, 512q)
    if packed:
        packed_arr = np.concatenate(packed, axis=0)
    else:
        packed_arr = np.zeros((128, 512), dtype=np.float32)
    return status, index, packed_arr.astype(BFNP)


def _cody_consts():
    c1 = float(np.float32(6.28125))
    r = np.float64(TWO_PI) - c1
    c2 = float(np.float32(r - np.remainder(r, 2.0 ** -24)))
    c3 = float(np.float32(np.float64(TWO_PI) - c1 - float(c2)))
    return c1, c2, c3


def _build(status, index, n_packed):
    from concourse import bacc, tile, mybir, bass_isa

    F32 = mybir.dt.float32
    BF16 = mybir.dt.bfloat16
    ACTF = mybir.ActivationFunctionType
    ALU = mybir.AluOpType
    X = mybir.AxisListType.X
    RADD = bass_isa.ReduceOp.add
    RG = [list(range(NCORES))]
    c1, c2, c3 = _cody_consts()

    nc = bacc.Bacc("TRN2", target_bir_lowering=False, debug=False,
                   num_devices=NCORES)

    xT = nc.dram_tensor("xT", [H, T], BF16, kind="ExternalInput")
    wqT = nc.dram_tensor("wqT", [H, QH * HD], F32, kind="ExternalInput")
    wkT = nc.dram_tensor("wkT", [H, HD], F32, kind="ExternalInput")
    wvT = nc.dram_tensor("wvT", [H, HD], F32, kind="ExternalInput")
    woT = nc.dram_tensor("woT", [H, MSH], F32, kind="ExternalInput")
    maskP = nc.dram_tensor("maskP", [n_packed * 128, 512], BF16,
                           kind="ExternalInput")
    pos = nc.dram_tensor("pos", [1, T], F32, kind="ExternalInput")
    outN = nc.dram_tensor("outN", [T, MSH], F32, kind="ExternalOutput")

    # [128, kt, col] views: element (p, k, c) = tensor[128k + p, c]
    wqR = wqT[:, :].rearrange("(k p) c -> p k c", p=128)
    wkR = wkT[:, :].rearrange("(k p) c -> p k c", p=128)
    wvR = wvT[:, :].rearrange("(k p) c -> p k c", p=128)
    woR = woT[:, :].rearrange("(k p) c -> p k c", p=128)
    xR = xT[:, :].rearrange("(k p) c -> p k c", p=128)
    mR = maskP[:, :].rearrange("(k p) c -> p k c", p=128)

    idn_c = nc.inline_tensor(np.eye(128, dtype=np.float32), name="idn_c")
    invf_np = (1.0 / THETA ** (np.arange(0, HD, 2, dtype=np.float32) / HD))
    invf_np = np.concatenate([invf_np, invf_np]).reshape(HD, 1)
    invf_c = nc.inline_tensor(invf_np.astype(np.float32), name="invf_c")

    with tile.TileContext(nc) as tc, \
         nc.allow_low_precision(reason="bf16 kernel"):
        with tc.tile_pool(name="cpool", bufs=1) as cpool, \
             tc.tile_pool(name="dbounce", bufs=1, space="DRAM") as dbounce:
            # DRAM bounce tiles (tracked by Tile for collective deps)
            ar_in = dbounce.tile([1, 8], F32, name="ar_in")
            ar_out = dbounce.tile([1, 8], F32, name="ar_out",
                                  addr_space="Shared")
            ar2_in = dbounce.tile([1, 8], F32, name="ar2_in")
            ar2_out = dbounce.tile([1, 8], F32, name="ar2_out",
                                   addr_space="Shared")
            agin = [[dbounce.tile([QH * HD, 512], BF16, name=f"agin{b}_{qc}")
                     for qc in range(SQC)] for b in range(B)]
            agout = [[dbounce.tile([H, 512], BF16, name=f"agout{b}_{qc}",
                                   addr_space="Shared") for qc in range(SQC)]
                     for b in range(B)]
            agoutR = [[agout[b][qc][:, :].rearrange("(k p) c -> p k c", p=128)
                       for qc in range(SQC)] for b in range(B)]

            idn_f = cpool.tile([128, 128], F32, name="idn_f")
            nc.sync.dma_start(out=idn_f[:], in_=idn_c[:, :])
            idn = cpool.tile([128, 128], BF16, name="idn")
            nc.scalar.copy(idn[:], idn_f[:])
            onesk = cpool.tile([128, 1], BF16, name="onesk")
            nc.vector.memset(onesk[:], 1.0)
            invf = cpool.tile([128, 1], F32, name="invf")
            nc.sync.dma_start(out=invf[:], in_=invf_c[:, :])
            cmag = cpool.tile([128, 1], F32, name="cmag")
            nc.vector.memset(cmag[:], C_MAGIC)
            cmag1 = cpool.tile([128, 1], F32, name="cmag1")
            nc.vector.memset(cmag1[:], C_MAGIC + 1.0)
            # broadcast scalar columns:
            #   bsc: 0=1/gq 1=1/gk 2=1/gv 3=c 4=1/c
            #   bsc2: 0=1/go 1=gv*go
            bsc = cpool.tile([128, 8], F32, name="bsc")
            bsc2 = cpool.tile([128, 2], F32, name="bsc2")

            # ---------------- pool allocation (LIFO by release time) ----
            # bottom: live until the end
            qkvp = tc.alloc_tile_pool(name="qkvp", bufs=1)
            qT_sb = [qkvp.tile([128, T], BF16, name=f"qT{h}")
                     for h in range(QH)]
            kT_sb = qkvp.tile([128, T], BF16, name="kT_sb")
            vT_sb = qkvp.tile([128, T], BF16, name="vT_sb")
            gacc = tc.alloc_tile_pool(name="gacc", bufs=1)
            qscr = tc.alloc_tile_pool(name="qscr", bufs=3)
            # released at end of phase 1 (LIFO: xpool, wstream, wqq_p, tabp)
            tabp = tc.alloc_tile_pool(name="tabp", bufs=1)
            cos_sb = tabp.tile([128, T], BF16, name="cos_sb")
            ss_sb = tabp.tile([128, T], BF16, name="ss_sb")

            # ---------------- RoPE tables (scratch freed before weights)
            with tc.tile_pool(name="rtab", bufs=2) as rtab:
                for tcn in range(NTC):
                    cs = slice(tcn * 512, (tcn + 1) * 512)
                    pchunk = rtab.tile([1, 512], F32, name=f"pos{tcn}",
                                       tag="pos")
                    nc.sync.dma_start(out=pchunk[:], in_=pos[0:1, cs])
                    pf = rtab.tile([128, 512], F32, name=f"pf{tcn}", tag="pf")
                    nc.gpsimd.partition_broadcast(pf[:], pchunk[:],
                                                  channels=128)
                    f_sb = rtab.tile([128, 512], F32, name=f"f{tcn}", tag="f")
                    nc.scalar.activation(f_sb[:], pf[:], ACTF.Copy,
                                         scale=invf[:])
                    k_sb = rtab.tile([128, 512], F32, name=f"kk{tcn}",
                                     tag="kk")
                    nc.vector.tensor_scalar(k_sb[:], f_sb[:], 1.0 / TWO_PI,
                                            C_MAGIC, ALU.mult, ALU.add)
                    nc.vector.tensor_scalar(k_sb[:], k_sb[:], C_MAGIC, None,
                                            ALU.subtract)
                    y_sb = rtab.tile([128, 512], F32, name=f"y{tcn}", tag="y")
                    nc.vector.scalar_tensor_tensor(
                        y_sb[:], k_sb[:], -c1, f_sb[:], ALU.mult, ALU.add)
                    nc.vector.scalar_tensor_tensor(
                        y_sb[:], k_sb[:], -c2, y_sb[:], ALU.mult, ALU.add)
                    nc.vector.scalar_tensor_tensor(
                        y_sb[:], k_sb[:], -c3, y_sb[:], ALU.mult, ALU.add)
                    nc.scalar.activation(ss_sb[0:64, cs], y_sb[0:64, :],
                                         ACTF.Sin, scale=-1.0)
                    nc.scalar.activation(ss_sb[64:128, cs], y_sb[64:128, :],
                                         ACTF.Sin)
                    yc = rtab.tile([128, 512], F32, name=f"yc{tcn}", tag="yc")
                    nc.vector.tensor_scalar(yc[:], y_sb[:],
                                            float(np.pi / 2), None, ALU.add)
                    m_sb = rtab.tile([128, 512], F32, name=f"mm{tcn}",
                                     tag="mm")
                    nc.vector.tensor_scalar(m_sb[:], yc[:], float(np.pi),
                                            None, ALU.is_gt)
                    nc.vector.scalar_tensor_tensor(
                        yc[:], m_sb[:], -TWO_PI, yc[:], ALU.mult, ALU.add)
                    nc.scalar.activation(cos_sb[:, cs], yc[:], ACTF.Sin)

            wqq_p = tc.alloc_tile_pool(name="wqq_p", bufs=1)
            wq_sb = wqq_p.tile([128, NKT, QH * HD], BF16, name="wq_sb")
            wk_sb = wqq_p.tile([128, NKT, HD], BF16, name="wk_sb")
            wv_sb = wqq_p.tile([128, NKT, HD], BF16, name="wv_sb")
            wkvp = tc.alloc_tile_pool(name="wkvp", bufs=1)
            wckv = wkvp.tile([128, NKT, 2 * HD], F32, name="wckv")
            # 8 slots: all wq chunks stay resident until quant reads them
            # in place (single HBM pass); wo pass-1 chunks then reuse slots
            wstream = tc.alloc_tile_pool(name="wstream", bufs=NKT // WCH)
            xpool = tc.alloc_tile_pool(name="xpool", bufs=2)

            # prefetch first x chunks on the ACT queue (lands immediately)
            xts0 = []
            for h2 in range(2):
                xt = xpool.tile([128, XCH, 512], BF16, name=f"x0_{h2}",
                                tag="xt")
                nc.scalar.dma_start(out=xt[:],
                                    in_=xR[:, h2 * XCH:(h2 + 1) * XCH,
                                           0:512])
                xts0.append(xt)

            # ---------------- weight pass 1: |w| sums (single HBM pass) --
            accq = gacc.tile([128, NKT], F32, name="accq")
            acck = gacc.tile([128, NKT], F32, name="acck")
            accv = gacc.tile([128, NKT], F32, name="accv")
            acco = gacc.tile([128, NKT], F32, name="acco")
            g4 = gacc.tile([128, 4], F32, name="g4")
            g4r = gacc.tile([128, 4], F32, name="g4r")
            go1 = gacc.tile([128, 1], F32, name="go1")
            go1r = gacc.tile([128, 1], F32, name="go1r")

            nc.sync.dma_start(out=wckv[:, :, 0:HD], in_=wkR[:, :, :])
            nc.sync.dma_start(out=wckv[:, :, HD:2 * HD], in_=wvR[:, :, :])
            NWCH = NKT // WCH
            wqstage = []
            for c in range(NWCH):
                wc = wstream.tile([128, WCH, 512], F32, name=f"p1q{c}",
                                  tag="wq")
                nc.sync.dma_start(out=wc[:],
                                  in_=wqR[:, c * WCH:(c + 1) * WCH, :])
                wqstage.append(wc)
                for i in range(WCH):
                    nc.vector.tensor_reduce(
                        accq[:, c * WCH + i:c * WCH + i + 1], wc[:, i, :], X,
                        ALU.add, apply_absolute_value=True)
            for i in range(NKT):
                nc.vector.tensor_reduce(acck[:, i:i + 1], wckv[:, i, 0:HD],
                                        X, ALU.add,
                                        apply_absolute_value=True)
                nc.vector.tensor_reduce(accv[:, i:i + 1],
                                        wckv[:, i, HD:2 * HD], X, ALU.add,
                                        apply_absolute_value=True)
            nc.vector.tensor_reduce(g4[:, 0:1], accq[:], X, ALU.add)
            nc.vector.tensor_reduce(g4[:, 1:2], acck[:], X, ALU.add)
            nc.vector.tensor_reduce(g4[:, 2:3], accv[:], X, ALU.add)
            nc.vector.memset(g4[:, 3:4], 0.0)
            nc.gpsimd.partition_all_reduce(g4r[:], g4[:], channels=128,
                                           reduce_op=RADD)
            gq_sb = gacc.tile([1, 8], F32, name="gq_sb")
            nc.vector.memset(gq_sb[:], 0.0)
            nc.scalar.copy(gq_sb[0:1, 0:4], g4r[0:1, 0:4])
            nc.sync.dma_start(out=ar_in[:], in_=gq_sb[:])
            nc.gpsimd.collective_compute(
                "AllReduce", ALU.add, replica_groups=RG,
                ins=[ar_in[:].opt()], outs=[ar_out[:].opt()])
            arq_sb = gacc.tile([1, 8], F32, name="arq_sb")
            nc.scalar.dma_start(out=arq_sb[:], in_=ar_out[:])

            # gamma math on partition 0 lanes
            gval = gacc.tile([1, 4], F32, name="gval")
            nc.vector.tensor_scalar(gval[0:1, 0:1], arq_sb[0:1, 0:1],
                                    1.0 / NQ, 1e-5, ALU.mult, ALU.add)
            nc.vector.tensor_scalar(gval[0:1, 1:3], arq_sb[0:1, 1:3],
                                    1.0 / NK, 1e-5, ALU.mult, ALU.add)
            gall = gacc.tile([1, 8], F32, name="gall")
            nc.vector.memset(gall[:], 0.0)
            nc.vector.reciprocal(gall[0:1, 0:3], gval[0:1, 0:3])
            nc.vector.tensor_mul(gall[0:1, 3:4], gval[0:1, 0:1],
                                 gval[0:1, 1:2])
            nc.vector.tensor_scalar(gall[0:1, 3:4], gall[0:1, 3:4],
                                    float(1.0 / np.sqrt(HD)), None, ALU.mult)
            nc.vector.reciprocal(gall[0:1, 4:5], gall[0:1, 3:4])
            nc.scalar.copy(gall[0:1, 5:6], gval[0:1, 2:3])
            nc.gpsimd.partition_broadcast(bsc[:], gall[:], channels=128)
            IQ, IK, IV = (bsc[:, i:i + 1] for i in range(3))
            CSC, ICS = bsc[:, 3:4], bsc[:, 4:5]

            # ---------------- quantized weights (pure ternary bf16) -----
            def quant_tile(src, dst, inv_ap, tagn, i, fsz):
                """dst_bf16 = clip(round_half_even(src/gamma), -1, 1).

                Round via the +C magic store (ACT); clip in magic space via
                two single-op DVE max/min against C-1/C+1; the final ACT
                Copy subtracts C exactly and converts to bf16. Avoids the
                slow DVE paths (dual-op ALU, bf16 output).
                """
                t = qscr.tile(list(src.shape), F32, name=f"qt{tagn}_{i}",
                              tag=f"qa{fsz}")
                nc.scalar.activation(t[:], src, ACTF.Identity,
                                     bias=cmag[:], scale=inv_ap)
                nc.vector.tensor_scalar(t[:], t[:], C_MAGIC - 1.0, None,
                                        ALU.max)
                nc.vector.tensor_scalar(t[:], t[:], C_MAGIC + 1.0, None,
                                        ALU.min)
                nc.scalar.activation(dst, t[:], ACTF.Copy, bias=-C_MAGIC)

            # quant straight from the resident staging (no second HBM pass),
            # interleaved in PE consumption order
            for c in range(NWCH):
                wc2 = wqstage[c]
                for i in range(WCH):
                    quant_tile(wc2[:, i, :], wq_sb[:, c * WCH + i, :], IQ,
                               f"q{c}", i, 512)
                for p in (2 * c, 2 * c + 1):
                    quant_tile(wckv[:, 2 * p:2 * p + 2, 0:HD],
                               wk_sb[:, 2 * p:2 * p + 2, :], IK,
                               f"k{c}", p, 256)
                    quant_tile(wckv[:, 2 * p:2 * p + 2, HD:2 * HD],
                               wv_sb[:, 2 * p:2 * p + 2, :], IV,
                               f"v{c}", p, 256)

            # ---------------- phase 1: QKV projections + RoPE -----------
            NXC = NKT // XCH

            # wo pass 1 tiles interleaved into phase 1 below
            def wo_pass1(c):
                wco = wstream.tile([128, WCH, 512], F32, name=f"p1o{c}",
                                   tag="wq")
                nc.sync.dma_start(out=wco[:],
                                  in_=woR[:, c * WCH:(c + 1) * WCH, :])
                for i in range(WCH):
                    nc.vector.tensor_reduce(
                        acco[:, c * WCH + i:c * WCH + i + 1], wco[:, i, :],
                        X, ALU.add, apply_absolute_value=True)

            with tc.tile_pool(name="rope", bufs=1) as rope, \
                 tc.tile_pool(name="p1", bufs=8, space="PSUM") as p1:
                for tcn in range(NTC):
                    cs = slice(tcn * 512, (tcn + 1) * 512)
                    pq = [p1.tile([128, 512], F32, name=f"pq{tcn}_{h}",
                                  tag="p1") for h in range(QH)]
                    pk = p1.tile([128, 512], F32, name=f"pk{tcn}", tag="p1")
                    pv = p1.tile([128, 512], F32, name=f"pv{tcn}", tag="p1")
                    for h2 in range(NXC):
                        if tcn == 0 and h2 < 2:
                            xt = xts0[h2]
                        else:
                            xt = xpool.tile([128, XCH, 512], BF16,
                                            name=f"x{tcn}_{h2}", tag="xt")
                            nc.sync.dma_start(
                                out=xt[:],
                                in_=xR[:, h2 * XCH:(h2 + 1) * XCH, cs])
                        for ktl in range(XCH):
                            kt = h2 * XCH + ktl
                            st, sp = (kt == 0), (kt == NKT - 1)
                            for h in range(QH):
                                nc.tensor.matmul(
                                    pq[h][:],
                                    wq_sb[:, kt, h * 128:(h + 1) * 128],
                                    xt[:, ktl, :], start=st, stop=sp,
                                    skip_group_check=True)
                            nc.tensor.matmul(
                                pk[:], wk_sb[:, kt, :], xt[:, ktl, :],
                                start=st, stop=sp, skip_group_check=True)
                            nc.tensor.matmul(
                                pv[:], wv_sb[:, kt, :], xt[:, ktl, :],
                                start=st, stop=sp, skip_group_check=True)

                    def rope_apply(psrc, dst_ap, tg):
                        m1 = rope.tile([128, 512], F32, name=f"m1{tg}",
                                       tag="m1")
                        nc.vector.tensor_mul(m1[:], psrc[:], cos_sb[:, cs])
                        m2 = rope.tile([128, 512], F32, name=f"m2{tg}",
                                       tag="m2")
                        nc.vector.tensor_mul(m2[0:64, :], psrc[64:128, :],
                                             ss_sb[0:64, cs])
                        nc.vector.tensor_mul(m2[64:128, :], psrc[0:64, :],
                                             ss_sb[64:128, cs])
                        nc.vector.tensor_add(dst_ap, m1[:], m2[:])

                    for h in range(QH):
                        rope_apply(pq[h], qT_sb[h][:, cs], f"_{tcn}_{h}")
                    rope_apply(pk, kT_sb[:, cs], f"k_{tcn}")
                    nc.scalar.copy(vT_sb[:, cs], pv[:])
                    # wo gamma pass interleaved (2 chunks per tcn)
                    wo_pass1(2 * tcn)
                    wo_pass1(2 * tcn + 1)

            # wo gamma: reduce + AllReduce + scalars
            nc.vector.tensor_reduce(go1[:, 0:1], acco[:], X, ALU.add)
            nc.gpsimd.partition_all_reduce(go1r[:], go1[:], channels=128,
                                           reduce_op=RADD)
            go_sb = gacc.tile([1, 8], F32, name="go_sb")
            nc.vector.memset(go_sb[:], 0.0)
            nc.scalar.copy(go_sb[0:1, 0:1], go1r[0:1, 0:1])
            nc.sync.dma_start(out=ar2_in[:], in_=go_sb[:])
            nc.gpsimd.collective_compute(
                "AllReduce", ALU.add, replica_groups=RG,
                ins=[ar2_in[:].opt()], outs=[ar2_out[:].opt()])
            aro_sb = gacc.tile([1, 8], F32, name="aro_sb")
            nc.gpsimd.dma_start(out=aro_sb[:], in_=ar2_out[:])
            gval2 = gacc.tile([1, 2], F32, name="gval2")
            nc.vector.tensor_scalar(gval2[0:1, 0:1], aro_sb[0:1, 0:1],
                                    1.0 / NO, 1e-5, ALU.mult, ALU.add)
            gall2 = gacc.tile([1, 2], F32, name="gall2")
            nc.vector.reciprocal(gall2[0:1, 0:1], gval2[0:1, 0:1])
            nc.vector.tensor_mul(gall2[0:1, 1:2], gval2[0:1, 0:1],
                                 gall[0:1, 5:6])
            nc.gpsimd.partition_broadcast(bsc2[:], gall2[:], channels=128)
            IO, OSC = bsc2[:, 0:1], bsc2[:, 1:2]

            xpool.release()
            wstream.release()
            wkvp.release()
            wqq_p.release()
            tabp.release()

            # ---------------- masks (scaled by 1/c) ---------------------
            wop = tc.alloc_tile_pool(name="wop", bufs=1)
            wo_sb = wop.tile([128, NKT, MSH], BF16, name="wo_sb")
            mall_p = tc.alloc_tile_pool(name="mall_p", bufs=1)
            mall = mall_p.tile([128, n_packed, 512], BF16, name="mall")
            nc.gpsimd.dma_start(out=mall[:], in_=mR[:, :, :])
            for mi in range(n_packed):
                nc.scalar.activation(mall[:, mi, :], mall[:, mi, :],
                                     ACTF.Copy, scale=ICS)

            # ---------------- attention + wo quant + o_proj -------------
            wostream = tc.alloc_tile_pool(name="wostream", bufs=1)

            def wo_quant(c):
                wc2 = wostream.tile([128, WCH, 512], F32, name=f"p2o{c}",
                                    tag="wo")
                nc.scalar.dma_start(out=wc2[:],
                                    in_=woR[:, c * WCH:(c + 1) * WCH, :])
                for i in range(WCH):
                    quant_tile(wc2[:, i, :], wo_sb[:, c * WCH + i, :], IO,
                               f"o{c}", i, 512)

            with tc.tile_pool(name="vnatp", bufs=2) as vnatp, \
                 tc.tile_pool(name="epool", bufs=34) as epool, \
                 tc.tile_pool(name="zrp", bufs=2) as zrp, \
                 tc.tile_pool(name="zbp", bufs=2) as zbp, \
                 tc.tile_pool(name="aop", bufs=2) as aop, \
                 tc.tile_pool(name="a3", bufs=3) as a3, \
                 tc.tile_pool(name="o3", bufs=2) as o3, \
                 tc.tile_pool(name="ps_s", bufs=2, space="PSUM") as ps_s, \
                 tc.tile_pool(name="ps_po", bufs=2, space="PSUM") as ps_po, \
                 tc.tile_pool(name="ps_z", bufs=1, space="PSUM") as ps_z, \
                 tc.tile_pool(name="ps_tr", bufs=1, space="PSUM") as ps_tr, \
                 tc.tile_pool(name="p3", bufs=2, space="PSUM") as p3:

                def build_vnat(b):
                    boff = b * S
                    vnat = vnatp.tile([128, S], BF16, name=f"vnat{b}",
                                      tag="vnat")
                    for kt in range(SKT):
                        ptr = ps_tr.tile([128, 128], BF16,
                                         name=f"ptr{b}_{kt}", tag="ptr")
                        nc.tensor.transpose(
                            ptr[:],
                            vT_sb[:, boff + kt * 128:boff + (kt + 1) * 128],
                            idn[:])
                        nc.scalar.copy(vnat[:, kt * 128:(kt + 1) * 128],
                                       ptr[:])
                    return vnat

                def attn_unit(b, qc, vnat):
                    """scores+exp for all 4 heads (exp-paced stream), then
                    softmax+AV per head-pair with a batched reciprocal."""
                    boff = b * S
                    kts = [kt for kt in range(SKT)
                           if status[b, kt, qc] != 1]
                    es = {}
                    for h in range(QH):
                        qsl = qT_sb[h][:, boff + qc * 512:
                                       boff + (qc + 1) * 512]
                        for kt in kts:
                            ps_ = ps_s.tile([128, 512], F32,
                                            name=f"s{b}{h}{qc}{kt}",
                                            tag="ps")
                            nc.tensor.matmul(
                                ps_[:],
                                kT_sb[:, boff + kt * 128:
                                      boff + (kt + 1) * 128],
                                qsl, start=True, stop=True,
                                skip_group_check=True)
                            if status[b, kt, qc] == 2:
                                mi = index[(b, kt, qc)]
                                nc.vector.tensor_add(ps_[:], ps_[:],
                                                     mall[:, mi, :])
                            e = epool.tile([128, 512], BF16,
                                           name=f"e{b}{h}{qc}{kt}", tag="e")
                            nc.scalar.activation(e[:], ps_[:], ACTF.Exp,
                                                 scale=CSC)
                            es[(h, kt)] = e
                    for h in range(QH):
                        pz = ps_z.tile([1, 512], F32,
                                       name=f"pz{b}{h}{qc}", tag="pz")
                        po = ps_po.tile([128, 512], F32,
                                        name=f"po{b}{h}{qc}", tag="po")
                        for i, kt in enumerate(kts):
                            fst, lst = (i == 0), (i == len(kts) - 1)
                            e = es[(h, kt)]
                            nc.tensor.matmul(pz[:], onesk[:], e[:],
                                             start=fst, stop=lst,
                                             skip_group_check=True)
                            nc.tensor.matmul(
                                po[:], vnat[:, kt * 128:(kt + 1) * 128],
                                e[:], start=fst, stop=lst,
                                skip_group_check=True)
                        # 1/z via exp(-ln z) on ACT (DVE reciprocal on a
                        # [1,512] lane is 2.8us; these are ~0.4us each)
                        zl = zrp.tile([1, 512], F32, name=f"zl{b}{h}{qc}",
                                      tag="zl")
                        nc.scalar.activation(zl[:], pz[:], ACTF.Ln)
                        zr = zrp.tile([1, 512], F32, name=f"zr{b}{h}{qc}",
                                      tag="zr")
                        nc.scalar.activation(zr[:], zl[:], ACTF.Exp,
                                             scale=-1.0)
                        zb = zbp.tile([128, 512], F32,
                                      name=f"zb{b}{h}{qc}", tag="zb")
                        nc.gpsimd.partition_broadcast(zb[:], zr[:],
                                                      channels=128)
                        aof = aop.tile([128, 512], F32,
                                       name=f"aof{b}{h}{qc}", tag="aof")
                        nc.vector.tensor_mul(aof[:], po[:], zb[:])
                        ao = aop.tile([128, 512], BF16,
                                      name=f"ao{b}{h}{qc}", tag="ao")
                        nc.scalar.activation(ao[:], aof[:], ACTF.Copy)
                        nc.sync.dma_start(
                            out=agin[b][qc][h * 128:(h + 1) * 128, :],
                            in_=ao[:])

                def ag_chunk(b, qc):
                    nc.gpsimd.collective_compute(
                        "AllGather", ALU.bypass, replica_groups=RG,
                        ins=[agin[b][qc][:, :].opt()],
                        outs=[agout[b][qc][:, :].opt()])

                def oproj(b, qc):
                    ch = b * 2 + qc
                    ats = []
                    for hf in range(2):
                        at = a3.tile([128, NKT // 2, 512], BF16,
                                     name=f"at{ch}_{hf}", tag="at")
                        nc.scalar.dma_start(
                            out=at[:],
                            in_=agoutR[b][qc][:, hf * 16:(hf + 1) * 16, :])
                        ats.append(at)
                    for tt in range(4):
                        pout = p3.tile([128, 512], F32, name=f"po3_{ch}{tt}",
                                       tag="pout")
                        for kt in range(NKT):
                            nc.tensor.matmul(
                                pout[:],
                                ats[kt // 16][:, kt % 16,
                                              tt * 128:(tt + 1) * 128],
                                wo_sb[:, kt, :], start=(kt == 0),
                                stop=(kt == NKT - 1), skip_group_check=True)
                        osb = o3.tile([128, 512], F32, name=f"osb{ch}{tt}",
                                      tag="osb")
                        nc.scalar.activation(osb[:], pout[:], ACTF.Copy,
                                             scale=OSC)
                        nc.sync.dma_start(
                            out=outN[ch * 512 + tt * 128:
                                     ch * 512 + (tt + 1) * 128, :],
                            in_=osb[:])

                # batch 0 attention, wo quant interleaved
                vnat0 = build_vnat(0)
                attn_unit(0, 0, vnat0)
                ag_chunk(0, 0)
                wo_quant(0)
                wo_quant(1)
                wo_quant(2)
                wo_quant(3)
                attn_unit(0, 1, vnat0)
                ag_chunk(0, 1)
                wo_quant(4)
                wo_quant(5)
                wo_quant(6)
                wo_quant(7)
                # batch 1 attention with o_proj chunks interleaved
                vnat1 = build_vnat(1)
                attn_unit(1, 0, vnat1)
                ag_chunk(1, 0)
                oproj(0, 0)
                attn_unit(1, 1, vnat1)
                ag_chunk(1, 1)
                oproj(0, 1)
                oproj(1, 0)
                oproj(1, 1)

            wostream.release()
            mall_p.release()
            wop.release()
            qscr.release()
            gacc.release()
            qkvp.release()

    nc.compile()
    return nc


def kernel(hidden_states, Wq, Wk, Wv, Wo, attention_mask, position_ids):
    from concourse.bass_utils import run_bass_kernel_spmd
    from concourse.bass_interp import get_hw_module

    hs = np.ascontiguousarray(np.asarray(hidden_states, dtype=np.float32))
    Wq = np.asarray(Wq, dtype=np.float32)
    Wk = np.asarray(Wk, dtype=np.float32)
    Wv = np.asarray(Wv, dtype=np.float32)
    Wo = np.asarray(Wo, dtype=np.float32)
    mask = np.asarray(attention_mask, dtype=np.float32)
    posf = np.ascontiguousarray(
        np.asarray(position_ids).reshape(1, T).astype(np.float32))

    status, index, packed = _classify_mask(mask)
    n_packed = packed.shape[0] // 128

    key = (status.tobytes(), n_packed)
    if key not in _cache:
        nc = _build(status, index, n_packed)
        nc.m = get_hw_module(nc.m)
        _cache[key] = nc
    nc = _cache[key]

    xTb = np.ascontiguousarray(hs.reshape(T, H).T.astype(BFNP))
    in_maps = []
    for c in range(NCORES):
        in_maps.append({
            "xT": xTb,
            "wqT": np.ascontiguousarray(
                Wq[c * QH * HD:(c + 1) * QH * HD, :].T),
            "wkT": np.ascontiguousarray(Wk[c * HD:(c + 1) * HD, :].T),
            "wvT": np.ascontiguousarray(Wv[c * HD:(c + 1) * HD, :].T),
            "woT": np.ascontiguousarray(Wo[c * MSH:(c + 1) * MSH, :].T),
            "maskP": packed,
            "pos": posf,
        })
    res = run_bass_kernel_spmd(nc, in_maps, core_ids=list(range(NCORES)),
                               trace=bool(os.environ.get("BITNET_TRACE")))
    global last_exec_time_ns
    last_exec_time_ns = res.exec_time_ns
    out = np.concatenate(
        [res.results[c]["outN"] for c in range(NCORES)], axis=1)  # (T, H)
    return np.ascontiguousarray(out).reshape(B, S, H).astype(np.float32)
